# revision 54
# baseline (speedup 1.0000x reference)
"""Llama GQA attention (B=2, S=2048, HID=2048, H=32, HKV=8, DH=64) on 8 TRN2 cores.

Sharding: tensor-parallel over heads. Core c owns q heads [4c, 4c+4) and kv
head c. One SPMD NEFF per run.

Fast causal path (bf16):
  1. Q/K/V projections in transposed layout with bf16 operands (full PE
     rate; fp32 matmuls run at half rate in fp32_mode=HIGH),
  2. RoPE via a signed-permutation matmul + DVE combines (bf16 in, f32
     combine, bf16 out),
  3. causal flash attention with scores kept transposed [k, q]; the two
     heads of a pair compute scores concurrently in PE row-groups 0/64 and
     share one [128, 2, 512] PSUM tile so a single ACT instruction
     exponentiates both (the ACT engine is the attention bottleneck:
     (N+352)-cycle cost per instruction, so batching the free dim matters),
  4. per-(batch, 512-token block) chunked AllGather of the normalized
     context (bf16), issued as soon as the block's attention finishes so
     collectives overlap compute,
  5. column-sharded o_proj per block, emitted as micro-tasks interleaved
     into the NEXT attention block's instruction stream so its matmuls
     fill PE bubbles while ACT works through the exps.
Host pre-transposes inputs (bf16) and assembles the 8 output slices.
"""
import sys

sys.path.insert(0, "/opt/trn_rl_repo")

import numpy as np

B, S, HID = 2, 2048, 2048
H, HKV, DH = 32, 8, 64
NC = 8
T = B * S
HPC = H // NC            # q heads per core (4)
CPC = HPC * DH           # ctx dims per core (256)
TB = 512                 # token block
KC = 128                 # k chunk
QBS = S // TB            # 4 q blocks per batch
SB_KC = S // KC          # 16 k chunks per batch
HCH = HID // 128         # 16 hid chunks
SCALE = DH ** -0.5
NEG = -1.0e30
RECIP_FAST = False
EXP_MERGE = True


def _build_fast():
    """Causal, bf16, chunked-AG, interleaved o_proj."""
    import concourse.mybir as mybir
    import concourse.tile as tile
    from concourse import bacc
    from concourse.masks import make_identity

    F32 = mybir.dt.float32
    BF16 = mybir.dt.bfloat16
    EXPF = mybir.ActivationFunctionType.Exp
    ADD = mybir.AluOpType.add
    MUL = mybir.AluOpType.mult

    nc = bacc.Bacc("TRN2", target_bir_lowering=False, debug=False, num_devices=NC)

    # host pre-arranged to partition-major layouts: one contiguous run per
    # partition per DMA (descriptor-issue time on Sync is ~10x cheaper than
    # the "(o p) m -> p o m" rearrange form)
    hT = nc.dram_tensor("hT", [128, B * QBS, HCH, TB], BF16,
                        kind="ExternalInput")
    wqT = nc.dram_tensor("wqT", [128, HCH, CPC], BF16, kind="ExternalInput")
    wkvT = nc.dram_tensor("wkvT", [128, HCH, 2 * DH], BF16,
                          kind="ExternalInput")
    woT = nc.dram_tensor("woT", [128, HCH, CPC], BF16, kind="ExternalInput")
    cosT = nc.dram_tensor("cosT", [DH, T], BF16, kind="ExternalInput")
    sinT = nc.dram_tensor("sinT", [DH, T], BF16, kind="ExternalInput")
    rot2p = nc.dram_tensor("rot2p", [128, 128], BF16, kind="ExternalInput")
    id64 = nc.dram_tensor("id64", [DH, DH], BF16, kind="ExternalInput")
    maskd = nc.dram_tensor("maskd", [128, 128], F32, kind="ExternalInput")
    outT = nc.dram_tensor("outT", [CPC, T], F32, kind="ExternalOutput")

    with tile.TileContext(nc) as tc:
        with tc.tile_pool(name="const", bufs=1) as cpool, \
             tc.tile_pool(name="big", bufs=1) as big, \
             tc.tile_pool(name="hstream", bufs=2) as hstream, \
             tc.tile_pool(name="cstream", bufs=2) as cstream, \
             tc.tile_pool(name="rope", bufs=3) as rope, \
             tc.tile_pool(name="attn", bufs=4) as attn, \
             tc.tile_pool(name="psM", bufs=1, space="PSUM") as psM, \
             tc.tile_pool(name="psS", bufs=2, space="PSUM") as psS, \
             tc.tile_pool(name="psC", bufs=1, space="PSUM") as psC, \
             tc.tile_pool(name="dram", bufs=1, space="DRAM") as dram:

            # ---- persistent SBUF (only wq/wkv loaded before first h) ----
            wq_sb = cpool.tile([128, HCH, CPC], BF16)
            nc.sync.dma_start(wq_sb[:, 0:HCH // 2, :], wqT[:, 0:HCH // 2, :])
            wkv_sb = cpool.tile([128, HCH, 2 * DH], BF16)
            nc.sync.dma_start(wkv_sb[:], wkvT[:])
            nc.sync.dma_start(wq_sb[:, HCH // 2:, :], wqT[:, HCH // 2:, :])
            # cos/sin duplicated across both 64-partition halves so one DVE
            # op covers a head PAIR ([128, TB] instead of 2x [64, TB])
            cos_sb = cpool.tile([128, T], BF16)
            sin_sb = cpool.tile([128, T], BF16)
            rot2_sb = cpool.tile([128, 128], BF16)
            id64_sb = cpool.tile([DH, DH], BF16)
            mk_sb = cpool.tile([128, 128], F32)
            wo_sb = cpool.tile([128, HCH, CPC], BF16)

            def load_consts():
                nc.sync.dma_start(rot2_sb[:], rot2p[:])
                nc.sync.dma_start(mk_sb[:], maskd[:])
                nc.sync.dma_start(id64_sb[:], id64[:])
                nc.sync.dma_start(cos_sb[0:DH, :], cosT[:])
                nc.sync.dma_start(cos_sb[DH:128, :], cosT[:])
                nc.sync.dma_start(sin_sb[0:DH, :], sinT[:])
                nc.sync.dma_start(sin_sb[DH:128, :], sinT[:])

            def load_wo():
                nc.sync.dma_start(wo_sb[:], woT[:])

            # ---- per-batch big activation buffers ----
            qT_sb = [[big.tile([128, S], BF16, tag=f"qT{b}{hp}", name=f"qT{b}{hp}")
                      for hp in range(2)] for b in range(B)]
            kT_sb = [big.tile([128, S], BF16, tag=f"kT{b}", name=f"kT{b}")
                     for b in range(B)]
            # 80-elem stride keeps each chunk 32B-aligned for DMA-transpose
            v_sb = [big.tile([128, SB_KC, 80], BF16, tag=f"v{b}", name=f"v{b}")
                    for b in range(B)]
            for b in range(B):
                nc.any.memset(v_sb[b][:, :, DH:DH + 1], 1.0)

            agi = [[dram.tile([CPC, TB], BF16, name=f"agi{b}_{qb}")
                    for qb in range(QBS)] for b in range(B)]
            ago = [[dram.tile([H * DH, TB], BF16, name=f"ago{b}_{qb}",
                              addr_space="Shared") for qb in range(QBS)]
                   for b in range(B)]
            # last block: per-hp split collective so hp0's AG overlaps hp1's
            # attention and only a half-size AG sits in the tail
            agi2 = [dram.tile([128, TB], BF16, name=f"agi2_{hp}")
                    for hp in range(2)]
            ago2 = [dram.tile([H * DH // 2, TB], BF16, name=f"ago2_{hp}",
                              addr_space="Shared") for hp in range(2)]

            pending_vraw = {}

            def proj_steps(b, qb, preload=None):
                """Generator emitting the q/k/v projection + RoPE for block
                (b, qb); yields between instruction groups so attn_block can
                interleave the emission into its own stream (keeps the PE
                queue fed during ACT-paced stretches)."""
                tb = b * QBS + qb
                gs = slice(tb * TB, (tb + 1) * TB)      # global token slice
                ls = slice(qb * TB, (qb + 1) * TB)      # slice within batch
                h = hstream.tile([128, HCH, TB], BF16, tag="h")
                # 4 sliced DMAs so the first matmuls start after 1/4 of h
                for hc in range(4):
                    nc.sync.dma_start(h[:, 4 * hc:4 * hc + 4, :],
                                      hT[:, tb, 4 * hc:4 * hc + 4, :])
                if preload is not None:
                    preload()
                yield
                # pass A: q heads 0,1 + kv
                pqA = psM.tile([128, TB], F32, tag="A", name=f"pqA_{tb}")
                pkv = psM.tile([128, TB], F32, tag="Bk", name=f"pkv_{tb}")
                for cc in range(HCH):
                    nc.tensor.matmul(pqA[:], wq_sb[:, cc, 0:128], h[:, cc, :],
                                     start=(cc == 0), stop=(cc == HCH - 1))
                    nc.tensor.matmul(pkv[:], wkv_sb[:, cc, :], h[:, cc, :],
                                     start=(cc == 0), stop=(cc == HCH - 1))
                    if cc % 2 == 1:
                        yield
                # evacuate pkv: V rows 0:64, K rows 64:128
                kraw = rope.tile([DH, TB], BF16, tag="kraw")
                nc.vector.tensor_copy(kraw[:], pkv[64:128, :])
                vraw = rope.tile([DH, TB], BF16, tag="vraw")
                nc.vector.tensor_copy(vraw[:], pkv[0:64, :])
                yield
                # evacuate q pair A early (frees psM tag A for the K rope)
                qrawA = rope.tile([128, TB], BF16, tag="qrawA")
                nc.vector.tensor_copy(qrawA[:], pqA[:])
                yield

                # rotate + combine, two heads per unit
                unit_idx = [0]

                def rope_unit(raw, dst, np_):
                    """raw/dst: [np_, TB] bf16 (np_=128 pair, 64 for K)."""
                    qcos = rope.tile([np_, TB], BF16, tag="cos")
                    nc.vector.tensor_tensor(qcos[:], raw[:], cos_sb[0:np_, gs],
                                            MUL)
                    # rot-MM lands in the proj banks (freed by the evacs)
                    tag = "A" if unit_idx[0] % 2 == 0 else "Bk"
                    unit_idx[0] += 1
                    rps = psM.tile([128, TB], F32, tag=tag,
                                   name=f"rps_{tb}_{id(raw)}")
                    nc.tensor.matmul(rps[0:np_, :], rot2_sb[0:np_, 0:np_],
                                     raw[:], start=True, stop=True)
                    yield
                    qsin = rope.tile([np_, TB], BF16, tag="sin")
                    nc.vector.tensor_tensor(qsin[:], rps[0:np_, :],
                                            sin_sb[0:np_, gs], MUL)
                    nc.vector.tensor_tensor(dst, qcos[:], qsin[:], ADD)
                    yield

                # K rope FIRST (tag A, freed by qrawA evac) + row-dup DMA:
                # early emission so the dup lands well before the next
                # block's x=1 score matmuls need rows 64:128
                yield from rope_unit(kraw, kT_sb[b][0:64, ls], DH)
                nc.sync.dma_start(kT_sb[b][64:128, ls], kT_sb[b][0:64, ls])
                yield
                # pass B: q heads 2,3 (tag Bk, freed by kraw/vraw evacs)
                pqB = psM.tile([128, TB], F32, tag="Bk", name=f"pqB_{tb}")
                for cc in range(HCH):
                    nc.tensor.matmul(pqB[:], wq_sb[:, cc, 128:256], h[:, cc, :],
                                     start=(cc == 0), stop=(cc == HCH - 1))
                    if cc % 2 == 1:
                        yield
                # V transpose deferred to emit_vtrans (PE-based; DMA-transpose
                # serializes against in-flight collectives and stalls Sync)
                pending_vraw[(b, qb)] = vraw
                qrawB = rope.tile([128, TB], BF16, tag="qrawB")
                nc.vector.tensor_copy(qrawB[:], pqB[:])
                yield
                yield from rope_unit(qrawA, qT_sb[b][0][:, ls], 128)
                yield from rope_unit(qrawB, qT_sb[b][1][:, ls], 128)

            def emit_vtrans(b, qb):
                """Transpose vraw [64, TB] -> v_sb [128tok, 4, 64d] via 4 PE
                matmuls against identity (vraw_chunk.T @ I), borrowing a
                score-pool PSUM buffer; evacuated by one strided DVE cast.
                Emitted at block boundaries (collective-free windows)."""
                vraw = pending_vraw.pop((b, qb))
                vt = psS.tile([128, 2, TB], F32, tag="s",
                              name=f"vt_{b}_{qb}")
                for i in range(TB // KC):
                    nc.tensor.matmul(vt[:, 0, i * DH:(i + 1) * DH],
                                     vraw[:, i * KC:(i + 1) * KC],
                                     id64_sb[:], start=True, stop=True)
                for i in range(TB // KC):
                    nc.vector.tensor_copy(
                        v_sb[b][:, qb * (TB // KC) + i, 0:DH],
                        vt[:, 0, i * DH:(i + 1) * DH])

            def oproj_tasks(b, qb):
                """Return list of thunks computing outT for (b, qb)."""
                gs = slice((b * QBS + qb) * TB, (b * QBS + qb + 1) * TB)
                tasks = []
                state = {}
                c = cstream.tile([128, HCH, TB], BF16, tag="c",
                                 name=f"c_{b}_{qb}")
                # 4 chunked loads (cheap issue, lets the first matmuls start
                # after a quarter); hoisted to the feed front by the caller
                for q4 in range(4):
                    tasks.append(lambda q4=q4: nc.sync.dma_start(
                        c[:, 4 * q4:4 * q4 + 4, :],
                        ago[b][qb][512 * q4:512 * (q4 + 1), :].rearrange(
                            "(o p) m -> p o m", p=128)))

                def mk_po(o):
                    state[o] = psM.tile([128, TB], F32,
                                        tag=("A" if o == 0 else "Bk"),
                                        name=f"po{o}_{b}_{qb}")

                def mm(o, cc, first, last):
                    nc.tensor.matmul(state[o][:],
                                     wo_sb[:, cc, o * 128:(o + 1) * 128],
                                     c[:, cc, :], start=first, stop=last)

                def fin(o):
                    osb = attn.tile([128, TB], F32, tag="osb")
                    nc.vector.tensor_copy(osb[:], state[o][:])
                    nc.sync.dma_start(outT[o * 128:(o + 1) * 128, gs], osb[:])

                for o in range(2):
                    tasks.append(lambda o=o: mk_po(o))
                    for j, cc in enumerate(range(HCH)):
                        tasks.append(lambda o=o, cc=cc, f=(j == 0),
                                     l=(j == HCH - 1): mm(o, cc, f, l))
                    tasks.append(lambda o=o: fin(o))
                return tasks

            def oproj_split_tasks(b, qb):
                """Last block's o_proj as (phaseA, phaseB): phaseA consumes
                the hp0 half-AllGather and drains during hp1's attention;
                phaseB (hp1 half + stores) is the only tail work."""
                gs = slice((b * QBS + qb) * TB, (b * QBS + qb + 1) * TB)
                state = {}
                # c[p, hp, o, m] = ago2[hp][128*o + p, m]; wo chunk cc
                # (global ctx rows [128cc,128cc+128)) maps to hp=cc%2, o=cc//2
                c = cstream.tile([128, 2, HCH // 2, TB], BF16, tag="c",
                                 name=f"c_{b}_{qb}")

                def load(hp):
                    nc.sync.dma_start(
                        c[:, hp, :, :],
                        ago2[hp][:].rearrange("(o p) m -> p o m", p=128))

                def mk_po(o):
                    state[o] = psM.tile([128, TB], F32,
                                        tag=("A" if o == 0 else "Bk"),
                                        name=f"po{o}_{b}_{qb}")

                def mm(o, cc, first, last):
                    nc.tensor.matmul(state[o][:],
                                     wo_sb[:, cc, o * 128:(o + 1) * 128],
                                     c[:, cc % 2, cc // 2, :],
                                     start=first, stop=last)

                def fin(o):
                    osb = attn.tile([128, TB], F32, tag="osb")
                    nc.vector.tensor_copy(osb[:], state[o][:])
                    nc.sync.dma_start(outT[o * 128:(o + 1) * 128, gs], osb[:])

                pa = [lambda: load(0), lambda: mk_po(0), lambda: mk_po(1)]
                for cc in range(0, HCH, 2):
                    for o in range(2):
                        pa.append(lambda o=o, cc=cc, f=(cc == 0):
                                  mm(o, cc, f, False))
                pb = [lambda: load(1)]
                for cc in range(1, HCH, 2):
                    for o in range(2):
                        pb.append(lambda o=o, cc=cc, l=(cc == HCH - 1):
                                  mm(o, cc, False, l))
                pb += [lambda: fin(0), lambda: fin(1)]
                return pa, pb

            def attn_block(b, qb, feed, split_ag=False, post_hp=None):
                def drain(n):
                    for _ in range(n):
                        if not feed.step():
                            return

                for hp in range(2):
                    ctxp = psC.tile([DH + 1, 2, TB], F32, tag="x",
                                    name=f"ctx_{b}_{qb}_{hp}")
                    kcs = list(range(4 * qb + 4))
                    for i, kc in enumerate(kcs):
                        c0 = 128 * (kc - 4 * qb) if kc >= 4 * qb else 0
                        nsl = slice(c0, TB)
                        qsl = slice(qb * TB + c0, (qb + 1) * TB)
                        sps = psS.tile([128, 2, TB], F32, tag="s",
                                       name=f"s_{b}_{qb}_{hp}_{kc}")
                        for x, off in ((0, 0), (1, 64)):
                            nc.tensor.matmul(
                                sps[:, x, nsl],
                                kT_sb[b][off:off + 64, kc * KC:(kc + 1) * KC],
                                qT_sb[b][hp][off:off + 64, qsl],
                                start=True, stop=True)
                        if kc >= 4 * qb:
                            for x in range(2):
                                nc.vector.tensor_tensor(
                                    sps[:, x, c0:c0 + 128],
                                    sps[:, x, c0:c0 + 128], mk_sb[:], ADD)
                        p = attn.tile([128, 2, TB], BF16, tag="p")
                        if EXP_MERGE:
                            nc.scalar.activation(p[:, :, nsl], sps[:, :, nsl],
                                                 EXPF, scale=SCALE)
                        else:
                            for x in range(2):
                                nc.scalar.activation(p[:, x, nsl],
                                                     sps[:, x, nsl],
                                                     EXPF, scale=SCALE)
                        for x in range(2):
                            nc.tensor.matmul(ctxp[:, x, nsl],
                                             v_sb[b][:, kc, 0:DH + 1],
                                             p[:, x, nsl], start=(i == 0),
                                             stop=(i == len(kcs) - 1),
                                             skip_group_check=True)
                        drain(2)
                    # normalize both heads of the pair; pack into one tile
                    ctxn = attn.tile([DH, 2, TB], BF16, tag="ctxn")
                    for x in range(2):
                        # fast approx reciprocal (~18 bits, 5x faster than
                        # DVE reciprocal; denom > 0 always so no edge cases).
                        # Stage the denominator row to SBUF via the Scalar
                        # engine (idle at block ends): the custom-DVE op
                        # misreads partition-64 PSUM sources directly.
                        rc = attn.tile([1, TB], F32, tag="rc")
                        dcp = attn.tile([1, TB], F32, tag="dcp")
                        nc.scalar.copy(dcp[:], ctxp[DH:DH + 1, x, :])
                        nc.vector.reciprocal_approx_fast(rc[:], dcp[:])
                        rb = attn.tile([DH, TB], F32, tag="rb")
                        nc.gpsimd.partition_broadcast(rb[:], rc[:])
                        nc.vector.tensor_tensor(ctxn[:, x, :],
                                                ctxp[0:DH, x, :], rb[:], MUL)
                        drain(2)
                    # two plain contiguous DMAs: the interleaved-rearrange
                    # single DMA cost 8.3us of Sync issue time
                    for x in range(2):
                        if split_ag:
                            nc.sync.dma_start(agi2[hp][64 * x:64 * x + 64, :],
                                              ctxn[:, x, :])
                        else:
                            r0 = 128 * hp + 64 * x
                            nc.sync.dma_start(agi[b][qb][r0:r0 + 64, :],
                                              ctxn[:, x, :])
                    if split_ag:
                        nc.gpsimd.collective_compute(
                            "AllGather", mybir.AluOpType.bypass,
                            replica_groups=[list(range(NC))],
                            ins=[agi2[hp].opt()], outs=[ago2[hp].opt()])
                    if post_hp is not None:
                        post_hp(hp)
                    drain(2)

            import concourse.mybir as _mybir

            class Feed:
                """Drain source for attn_block: first advances the next
                block's proj emission (so its PSUM-bank claims stay ahead of
                o_proj's), then pops queued o_proj tasks."""

                def __init__(self, gen, tasks):
                    self.gen = gen
                    self.tasks = tasks

                def step(self):
                    if self.gen is not None:
                        try:
                            next(self.gen)
                            return True
                        except StopIteration:
                            self.gen = None
                    if self.tasks:
                        self.tasks.pop(0)()
                        return True
                    return False

            # software pipeline: proj(n+1) emission is interleaved INTO
            # attn(n)'s drain slots (ahead of o_proj tasks), so the PE queue
            # always holds ready work while attn(n) waits on ACT.
            # o_proj(n) drains during attn(n+3) mid-stream; the last two
            # attention blocks each drain two o_projs so only the final
            # (split-AG) o_proj remains in the tail.
            blocks = [(b, qb) for b in range(B) for qb in range(QBS)]
            queue = []             # FIFO of oproj task lists
            for _ in proj_steps(*blocks[0], preload=load_consts):
                pass
            emit_vtrans(*blocks[0])
            load_wo()
            for i, (b, qb) in enumerate(blocks):
                gen = (proj_steps(*blocks[i + 1])
                       if i + 1 < len(blocks) else None)
                last = (i == len(blocks) - 1)
                npop = (2 if i >= 6 else (1 if i >= 3 else 0))
                loads, tasks = [], []
                while queue and npop > 0:
                    lst = queue.pop(0)
                    loads += lst[:4]     # hoist the 4 chunked c-loads
                    tasks += lst[4:]
                    npop -= 1
                tasks = loads + tasks
                feed = Feed(gen, tasks)
                if last:
                    pa, pb = oproj_split_tasks(b, qb)

                    def post_hp(hp):
                        if hp == 0:
                            feed.tasks += pa
                        else:
                            # issue the hp1 c-load now: the DMA parks on the
                            # AllGather semaphore and fires the moment the
                            # gathered data lands
                            pb.pop(0)()
                    attn_block(b, qb, feed, split_ag=True, post_hp=post_hp)
                else:
                    attn_block(b, qb, feed)
                # finish proj(n+1) emission (incl. its PE V-transpose)
                # BEFORE the AllGather so nothing serializes behind it
                if feed.gen is not None:
                    for _ in feed.gen:
                        pass
                    feed.gen = None
                if i + 1 < len(blocks):
                    emit_vtrans(*blocks[i + 1])
                if not last:
                    nc.gpsimd.collective_compute(
                        "AllGather", _mybir.AluOpType.bypass,
                        replica_groups=[list(range(NC))],
                        ins=[agi[b][qb].opt()], outs=[ago[b][qb].opt()])
                while feed.step():
                    pass
                if not last:
                    queue.append(oproj_tasks(b, qb))
                else:
                    for t in pb:
                        t()

    nc.compile()
    return nc


def _build_general():
    """Fallback f32 path for non-causal masks (full mask streamed)."""
    import concourse.mybir as mybir
    import concourse.tile as tile
    from concourse import bacc
    from concourse.masks import make_identity

    F32 = mybir.dt.float32
    F32R = mybir.dt.float32r
    EXPF = mybir.ActivationFunctionType.Exp
    ADD = mybir.AluOpType.add
    MUL = mybir.AluOpType.mult

    nc = bacc.Bacc("TRN2", target_bir_lowering=False, debug=False, num_devices=NC)

    hT = nc.dram_tensor("hT", [HID, T], F32R, kind="ExternalInput")
    wqT = nc.dram_tensor("wqT", [HID, CPC], F32R, kind="ExternalInput")
    wkvT = nc.dram_tensor("wkvT", [HID, 2 * DH], F32R, kind="ExternalInput")
    woT = nc.dram_tensor("woT", [H * DH, CPC], F32R, kind="ExternalInput")
    cosT = nc.dram_tensor("cosT", [DH, T], F32, kind="ExternalInput")
    sinT = nc.dram_tensor("sinT", [DH, T], F32, kind="ExternalInput")
    rotp = nc.dram_tensor("rotp", [DH, DH], F32R, kind="ExternalInput")
    maskg = nc.dram_tensor("maskg", [S, S], F32, kind="ExternalInput")
    outT = nc.dram_tensor("outT", [CPC, T], F32, kind="ExternalOutput")

    with tile.TileContext(nc) as tc:
        with tc.tile_pool(name="const", bufs=1) as cpool, \
             tc.tile_pool(name="big", bufs=1) as big, \
             tc.tile_pool(name="stream", bufs=3) as stream, \
             tc.tile_pool(name="rope", bufs=2) as rope, \
             tc.tile_pool(name="attn", bufs=3) as attn, \
             tc.tile_pool(name="psM", bufs=1, space="PSUM") as psM, \
             tc.tile_pool(name="psS", bufs=3, space="PSUM") as psS, \
             tc.tile_pool(name="psC", bufs=1, space="PSUM") as psC, \
             tc.tile_pool(name="dram", bufs=1, space="DRAM") as dram:

            wq_sb = cpool.tile([128, HCH, CPC], F32R)
            nc.sync.dma_start(wq_sb[:], wqT[:].rearrange("(o p) m -> p o m", p=128))
            wkv_sb = cpool.tile([128, HCH, 2 * DH], F32R)
            nc.sync.dma_start(wkv_sb[:], wkvT[:].rearrange("(o p) m -> p o m", p=128))
            wo_sb = cpool.tile([128, HCH, CPC], F32R)
            nc.sync.dma_start(wo_sb[:], woT[:].rearrange("(o p) m -> p o m", p=128))
            cos_sb = cpool.tile([DH, T], F32)
            nc.sync.dma_start(cos_sb[:], cosT[:])
            sin_sb = cpool.tile([DH, T], F32)
            nc.sync.dma_start(sin_sb[:], sinT[:])
            rot_sb = cpool.tile([DH, DH], F32R)
            nc.sync.dma_start(rot_sb[:], rotp[:])
            onesc_f = cpool.tile([128, SB_KC, 1], F32)
            nc.any.memset(onesc_f[:], 1.0)
            ident = cpool.tile([DH, DH], F32)
            make_identity(nc, ident)

            qT_sb = [[big.tile([128, S], F32R, tag=f"qT{b}{hp}", name=f"qT{b}{hp}")
                      for hp in range(2)] for b in range(B)]
            kT_sb = [big.tile([128, S], F32R, tag=f"kT{b}", name=f"kT{b}")
                     for b in range(B)]
            v_sb = [big.tile([128, SB_KC, DH + 1], F32R, tag=f"v{b}", name=f"v{b}")
                    for b in range(B)]
            for b in range(B):
                nc.vector.tensor_copy(v_sb[b][:, :, DH:DH + 1], onesc_f[:])

            ag_in = [dram.tile([CPC, S], F32R, name=f"agi{b}") for b in range(B)]
            ag_out = [dram.tile([H * DH, S], F32R, name=f"ago{b}",
                                addr_space="Shared") for b in range(B)]

            def proj_block(b, qb):
                tb = b * QBS + qb
                gs = slice(tb * TB, (tb + 1) * TB)
                ls = slice(qb * TB, (qb + 1) * TB)
                pq = [psM.tile([128, TB], F32, tag=f"mm{hp}", name=f"pq{hp}_{tb}")
                      for hp in range(2)]
                pkv = psM.tile([128, TB], F32, tag="mmkv")
                for cc in range(HCH):
                    h_sb = stream.tile([128, TB], F32R, tag="h")
                    nc.sync.dma_start(h_sb[:], hT[cc * 128:(cc + 1) * 128, gs])
                    for hp in range(2):
                        nc.tensor.matmul(pq[hp][:],
                                         wq_sb[:, cc, hp * 128:(hp + 1) * 128],
                                         h_sb[:], start=(cc == 0),
                                         stop=(cc == HCH - 1))
                    nc.tensor.matmul(pkv[:], wkv_sb[:, cc, :], h_sb[:],
                                     start=(cc == 0), stop=(cc == HCH - 1))
                for h in range(HPC):
                    hp, hh = h // 2, 64 * (h % 2)
                    src = pq[hp][hh:hh + 64, :]
                    qraw = rope.tile([DH, TB], F32R, tag="raw")
                    nc.vector.tensor_copy(qraw[:], src)
                    qcos = rope.tile([DH, TB], F32, tag="cos")
                    nc.vector.tensor_tensor(qcos[:], qraw[:].bitcast(F32),
                                            cos_sb[:, gs], MUL)
                    rps = psS.tile([DH, TB], F32, tag="s")
                    nc.tensor.matmul(rps[:], rot_sb[:], qraw[:], start=True,
                                     stop=True)
                    qsin = rope.tile([DH, TB], F32, tag="sin")
                    nc.vector.tensor_tensor(qsin[:], rps[:], sin_sb[:, gs], MUL)
                    if hh == 0:
                        nc.vector.tensor_tensor(qT_sb[b][hp][0:64, ls],
                                                qcos[:], qsin[:], ADD)
                    else:
                        qfin = rope.tile([DH, TB], F32R, tag="fin")
                        nc.vector.tensor_tensor(qfin[:], qcos[:], qsin[:], ADD)
                        nc.sync.dma_start(qT_sb[b][hp][64:128, ls], qfin[:])
                ksrc = pkv[64:128, :]
                kraw = rope.tile([DH, TB], F32R, tag="raw")
                nc.vector.tensor_copy(kraw[:], ksrc)
                kcos = rope.tile([DH, TB], F32, tag="cos")
                nc.vector.tensor_tensor(kcos[:], kraw[:].bitcast(F32),
                                        cos_sb[:, gs], MUL)
                krps = psS.tile([DH, TB], F32, tag="s")
                nc.tensor.matmul(krps[:], rot_sb[:], kraw[:], start=True, stop=True)
                ksin = rope.tile([DH, TB], F32, tag="sin")
                nc.vector.tensor_tensor(ksin[:], krps[:], sin_sb[:, gs], MUL)
                nc.vector.tensor_tensor(kT_sb[b][0:64, ls], kcos[:], ksin[:], ADD)
                nc.sync.dma_start(kT_sb[b][64:128, ls], kT_sb[b][0:64, ls])
                vraw = rope.tile([DH, TB], F32, tag="vraw")
                nc.vector.tensor_copy(vraw[:], pkv[0:64, :])
                for i in range(TB // KC):
                    vtp = psS.tile([128, DH], F32, tag="s")
                    nc.tensor.transpose(vtp[:], vraw[:, i * KC:(i + 1) * KC],
                                        ident[:])
                    nc.vector.tensor_copy(v_sb[b][:, qb * (TB // KC) + i, 0:DH],
                                          vtp[:])

            def attn_block(b, qb):
                for hp in range(2):
                    kcs = list(range(SB_KC))
                    ctxp = [psC.tile([DH + 1, TB], F32, tag=f"ctx{x}",
                                     name=f"ctx{x}_{b}_{qb}_{hp}")
                            for x in range(2)]
                    for i, kc in enumerate(kcs):
                        sps = [psS.tile([128, TB], F32, tag="s",
                                        name=f"s{x}_{b}_{qb}_{hp}_{kc}")
                               for x in range(2)]
                        for x, hh in enumerate((0, 64)):
                            nc.tensor.matmul(
                                sps[x][:],
                                kT_sb[b][hh:hh + 64, kc * KC:(kc + 1) * KC],
                                qT_sb[b][hp][hh:hh + 64,
                                             qb * TB:(qb + 1) * TB],
                                start=True, stop=True)
                        for x in range(2):
                            mg = attn.tile([128, TB], F32, tag="mg")
                            nc.sync.dma_start(
                                mg[:], maskg[kc * KC:(kc + 1) * KC,
                                             qb * TB:(qb + 1) * TB])
                            nc.vector.tensor_tensor(sps[x][:], sps[x][:],
                                                    mg[:], ADD)
                            p_sb = attn.tile([128, TB], F32R, tag="p")
                            nc.scalar.activation(p_sb[:], sps[x][:],
                                                 EXPF, scale=SCALE)
                            nc.tensor.matmul(ctxp[x][:], v_sb[b][:, kc, :],
                                             p_sb[:], start=(i == 0),
                                             stop=(i == len(kcs) - 1),
                                             skip_group_check=True)
                    for x in range(2):
                        h = 2 * hp + x
                        rc = attn.tile([1, TB], F32R, tag="rc")
                        with nc.allow_low_precision(reason="f32r rounding ~1e-4"):
                            nc.vector.reciprocal(rc[:], ctxp[x][DH:DH + 1, :])
                        rb = attn.tile([DH, TB], F32R, tag="rb")
                        nc.gpsimd.partition_broadcast(rb[:], rc[:])
                        ctxn = attn.tile([DH, TB], F32R, tag="ctxn")
                        nc.vector.tensor_tensor(ctxn[:], ctxp[x][0:DH, :],
                                                rb[:].bitcast(F32), MUL)
                        nc.sync.dma_start(
                            ag_in[b][h * DH:(h + 1) * DH,
                                     qb * TB:(qb + 1) * TB],
                            ctxn[:])

            for b in range(B):
                for qb in range(QBS):
                    proj_block(b, qb)
                    attn_block(b, qb)
                import concourse.mybir as _mybir
                nc.gpsimd.collective_compute(
                    "AllGather", _mybir.AluOpType.bypass,
                    replica_groups=[list(range(NC))],
                    ins=[ag_in[b].opt()], outs=[ag_out[b].opt()])

            for b in range(B):
                for qb in range(QBS):
                    ls = slice(qb * TB, (qb + 1) * TB)
                    gs = slice((b * QBS + qb) * TB, (b * QBS + qb + 1) * TB)
                    po = [psM.tile([128, TB], F32, tag=f"mm{o}",
                                   name=f"po{o}_{b}_{qb}")
                          for o in range(2)]
                    for cc in range(HCH):
                        c_sb = stream.tile([128, TB], F32R, tag="c")
                        nc.sync.dma_start(
                            c_sb[:], ag_out[b][cc * 128:(cc + 1) * 128, ls])
                        for o in range(2):
                            nc.tensor.matmul(po[o][:],
                                             wo_sb[:, cc, o * 128:(o + 1) * 128],
                                             c_sb[:], start=(cc == 0),
                                             stop=(cc == HCH - 1))
                    for o in range(2):
                        o_sb = stream.tile([128, TB], F32, tag="o")
                        nc.vector.tensor_copy(o_sb[:], po[o][:])
                        nc.sync.dma_start(outT[o * 128:(o + 1) * 128, gs],
                                          o_sb[:])

    nc.compile()
    return nc


def _host_inputs(hidden_states, cos, sin, attention_mask, Wq, Wk, Wv, Wo,
                 causal):
    hT = np.ascontiguousarray(hidden_states.reshape(T, HID).T)
    cosT = np.ascontiguousarray(cos.reshape(T, DH).T)
    sinT = np.ascontiguousarray(sin.reshape(T, DH).T)
    # rot_half as a signed permutation: rot[d] = -x[d+32] (d<32), +x[d-32]
    p64 = np.zeros((DH, DH), np.float32)
    for m in range(32):
        p64[m + 32, m] = -1.0
        p64[m, m + 32] = 1.0
    WqT = np.ascontiguousarray(Wq.T)      # [HID, H*DH]
    WkT = np.ascontiguousarray(Wk.T)      # [HID, HKV*DH]
    WvT = np.ascontiguousarray(Wv.T)
    WoT = np.ascontiguousarray(Wo.T)      # [H*DH, HID]

    if causal:
        import ml_dtypes
        bf = ml_dtypes.bfloat16
        # partition-major pre-arrangements: X[o*128+p, m] -> X3[p, o, m]
        # (and per-block for hT) so each device DMA is one contiguous run
        # per partition
        hT = np.ascontiguousarray(
            hT.astype(bf).reshape(HCH, 128, B * S // 512, 512)
            .transpose(1, 2, 0, 3))
        WqT, WkT, WvT, WoT = (w.astype(bf) for w in (WqT, WkT, WvT, WoT))
        cosT = cosT.astype(bf)
        sinT = sinT.astype(bf)
        # block-diag rot for head-paired RoPE ([128,128]); upper-left 64x64
        # block doubles as the single-head (K) rot matrix
        p128 = np.zeros((128, 128), np.float32)
        p128[0:DH, 0:DH] = p64
        p128[DH:128, DH:128] = p64
        p128 = p128.astype(bf)
    def pmaj(w):
        """[HCH*128, m] -> [128, HCH, m] partition-major (causal path)."""
        if not causal:
            return np.ascontiguousarray(w)
        return np.ascontiguousarray(
            w.reshape(HCH, 128, w.shape[1]).transpose(1, 0, 2))

    ins = []
    for c in range(NC):
        d = {
            "hT": hT,
            "wqT": pmaj(WqT[:, c * CPC:(c + 1) * CPC]),
            "wkvT": pmaj(
                np.concatenate([WvT[:, c * DH:(c + 1) * DH],
                                WkT[:, c * DH:(c + 1) * DH]], axis=1)),
            "woT": pmaj(WoT[:, c * CPC:(c + 1) * CPC]),
            "cosT": cosT, "sinT": sinT,
        }
        if causal:
            d["rot2p"] = p128
            d["id64"] = np.eye(DH, dtype=np.float32).astype(p128.dtype)
        else:
            d["rotp"] = p64
        if causal:
            i = np.arange(128, dtype=np.float32)[:, None]
            cc = np.arange(128, dtype=np.float32)[None, :]
            d["maskd"] = np.where(cc < i, NEG, 0.0).astype(np.float32)
        else:
            m = attention_mask[0, 0].astype(np.float32)
            d["maskg"] = np.ascontiguousarray(m.T) * np.float32(1.0 / SCALE)
        ins.append(d)
    return ins


def _is_causal(attention_mask):
    if attention_mask.shape != (1, 1, S, S):
        return False
    m = attention_mask[0, 0]
    neg = np.finfo(np.float32).min
    tril = np.tril(np.ones((S, S), dtype=bool))
    expect = np.where(tril, np.float32(0.0), np.float32(neg))
    return np.array_equal(m, expect)


_CACHE = {}


def _get_nc(causal):
    if causal not in _CACHE:
        _CACHE[causal] = _build_fast() if causal else _build_general()
    return _CACHE[causal]


def kernel(**inputs) -> np.ndarray:
    from concourse.bass_utils import run_bass_kernel_spmd

    hidden_states = np.asarray(inputs["hidden_states"], np.float32)
    cos = np.asarray(inputs["cos"], np.float32)
    sin = np.asarray(inputs["sin"], np.float32)
    attention_mask = np.asarray(inputs["attention_mask"], np.float32)
    Wq = np.asarray(inputs["Wq"], np.float32)
    Wk = np.asarray(inputs["Wk"], np.float32)
    Wv = np.asarray(inputs["Wv"], np.float32)
    Wo = np.asarray(inputs["Wo"], np.float32)

    causal = _is_causal(attention_mask)
    nc = _get_nc(causal)
    ins = _host_inputs(hidden_states, cos, sin, attention_mask,
                       Wq, Wk, Wv, Wo, causal)
    res = run_bass_kernel_spmd(nc, ins, core_ids=list(range(NC)))
    outT = np.concatenate([res.results[c]["outT"] for c in range(NC)], axis=0)
    return np.ascontiguousarray(outT.T).reshape(B, S, HID)



# revision 55
# speedup vs baseline: 1.0188x; 1.0188x over previous
"""Llama GQA attention (B=2, S=2048, HID=2048, H=32, HKV=8, DH=64) on 8 TRN2 cores.

Sharding: tensor-parallel over heads. Core c owns q heads [4c, 4c+4) and kv
head c. One SPMD NEFF per run.

Fast causal path (bf16):
  1. Q/K/V projections in transposed layout with bf16 operands (full PE
     rate; fp32 matmuls run at half rate in fp32_mode=HIGH),
  2. RoPE via a signed-permutation matmul + DVE combines (bf16 in, f32
     combine, bf16 out),
  3. causal flash attention with scores kept transposed [k, q]; the two
     heads of a pair compute scores concurrently in PE row-groups 0/64 and
     share one [128, 2, 512] PSUM tile so a single ACT instruction
     exponentiates both (the ACT engine is the attention bottleneck:
     (N+352)-cycle cost per instruction, so batching the free dim matters),
  4. per-(batch, 512-token block) chunked AllGather of the normalized
     context (bf16), issued as soon as the block's attention finishes so
     collectives overlap compute,
  5. column-sharded o_proj per block, emitted as micro-tasks interleaved
     into the NEXT attention block's instruction stream so its matmuls
     fill PE bubbles while ACT works through the exps.
Host pre-transposes inputs (bf16) and assembles the 8 output slices.
"""
import sys

sys.path.insert(0, "/opt/trn_rl_repo")

import numpy as np

B, S, HID = 2, 2048, 2048
H, HKV, DH = 32, 8, 64
NC = 8
T = B * S
HPC = H // NC            # q heads per core (4)
CPC = HPC * DH           # ctx dims per core (256)
TB = 512                 # token block
KC = 128                 # k chunk
QBS = S // TB            # 4 q blocks per batch
SB_KC = S // KC          # 16 k chunks per batch
HCH = HID // 128         # 16 hid chunks
SCALE = DH ** -0.5
NEG = -1.0e30
RECIP_FAST = False
EXP_MERGE = True


def _build_fast():
    """Causal, bf16, chunked-AG, interleaved o_proj."""
    import concourse.mybir as mybir
    import concourse.tile as tile
    from concourse import bacc
    from concourse.masks import make_identity

    F32 = mybir.dt.float32
    BF16 = mybir.dt.bfloat16
    EXPF = mybir.ActivationFunctionType.Exp
    ADD = mybir.AluOpType.add
    MUL = mybir.AluOpType.mult

    nc = bacc.Bacc("TRN2", target_bir_lowering=False, debug=False, num_devices=NC)

    # host pre-arranged to partition-major layouts: one contiguous run per
    # partition per DMA (descriptor-issue time on Sync is ~10x cheaper than
    # the "(o p) m -> p o m" rearrange form)
    hT = nc.dram_tensor("hT", [128, B * QBS, HCH, TB], BF16,
                        kind="ExternalInput")
    wqT = nc.dram_tensor("wqT", [128, HCH, CPC], BF16, kind="ExternalInput")
    wkvT = nc.dram_tensor("wkvT", [128, HCH, 2 * DH], BF16,
                          kind="ExternalInput")
    woT = nc.dram_tensor("woT", [128, HCH, CPC], BF16, kind="ExternalInput")
    cosT = nc.dram_tensor("cosT", [DH, T], BF16, kind="ExternalInput")
    sinT = nc.dram_tensor("sinT", [DH, T], BF16, kind="ExternalInput")
    rot2p = nc.dram_tensor("rot2p", [128, 128], BF16, kind="ExternalInput")
    id64 = nc.dram_tensor("id64", [DH, DH], BF16, kind="ExternalInput")
    maskd = nc.dram_tensor("maskd", [128, 128], F32, kind="ExternalInput")
    outT = nc.dram_tensor("outT", [CPC, T], F32, kind="ExternalOutput")

    with tile.TileContext(nc) as tc:
        with tc.tile_pool(name="const", bufs=1) as cpool, \
             tc.tile_pool(name="big", bufs=1) as big, \
             tc.tile_pool(name="hstream", bufs=2) as hstream, \
             tc.tile_pool(name="cstream", bufs=2) as cstream, \
             tc.tile_pool(name="rope", bufs=3) as rope, \
             tc.tile_pool(name="attn", bufs=4) as attn, \
             tc.tile_pool(name="psM", bufs=1, space="PSUM") as psM, \
             tc.tile_pool(name="psS", bufs=2, space="PSUM") as psS, \
             tc.tile_pool(name="psC", bufs=1, space="PSUM") as psC, \
             tc.tile_pool(name="dram", bufs=1, space="DRAM") as dram:

            # ---- persistent SBUF (only wq/wkv loaded before first h) ----
            wq_sb = cpool.tile([128, HCH, CPC], BF16)
            nc.sync.dma_start(wq_sb[:, 0:HCH // 2, :], wqT[:, 0:HCH // 2, :])
            wkv_sb = cpool.tile([128, HCH, 2 * DH], BF16)
            nc.sync.dma_start(wkv_sb[:], wkvT[:])
            nc.sync.dma_start(wq_sb[:, HCH // 2:, :], wqT[:, HCH // 2:, :])
            # cos/sin duplicated across both 64-partition halves so one DVE
            # op covers a head PAIR ([128, TB] instead of 2x [64, TB])
            cos_sb = cpool.tile([128, T], BF16)
            sin_sb = cpool.tile([128, T], BF16)
            rot2_sb = cpool.tile([128, 128], BF16)
            id64_sb = cpool.tile([DH, DH], BF16)
            mk_sb = cpool.tile([128, 128], F32)
            wo_sb = cpool.tile([128, HCH, CPC], BF16)

            def load_consts():
                nc.sync.dma_start(rot2_sb[:], rot2p[:])
                nc.sync.dma_start(mk_sb[:], maskd[:])
                nc.sync.dma_start(id64_sb[:], id64[:])
                nc.sync.dma_start(cos_sb[0:DH, :], cosT[:])
                nc.sync.dma_start(cos_sb[DH:128, :], cosT[:])
                nc.sync.dma_start(sin_sb[0:DH, :], sinT[:])
                nc.sync.dma_start(sin_sb[DH:128, :], sinT[:])

            def load_wo():
                nc.sync.dma_start(wo_sb[:], woT[:])

            # ---- per-batch big activation buffers ----
            qT_sb = [[big.tile([128, S], BF16, tag=f"qT{b}{hp}", name=f"qT{b}{hp}")
                      for hp in range(2)] for b in range(B)]
            kT_sb = [big.tile([128, S], BF16, tag=f"kT{b}", name=f"kT{b}")
                     for b in range(B)]
            # 80-elem stride keeps each chunk 32B-aligned for DMA-transpose
            v_sb = [big.tile([128, SB_KC, 80], BF16, tag=f"v{b}", name=f"v{b}")
                    for b in range(B)]
            for b in range(B):
                nc.any.memset(v_sb[b][:, :, DH:DH + 1], 1.0)

            agi = [[dram.tile([CPC, TB], BF16, name=f"agi{b}_{qb}")
                    for qb in range(QBS)] for b in range(B)]
            ago = [[dram.tile([H * DH, TB], BF16, name=f"ago{b}_{qb}",
                              addr_space="Shared") for qb in range(QBS)]
                   for b in range(B)]
            # last block: per-hp split collective so hp0's AG overlaps hp1's
            # attention and only a half-size AG sits in the tail
            agi2 = [dram.tile([128, TB], BF16, name=f"agi2_{hp}")
                    for hp in range(2)]
            ago2 = [dram.tile([H * DH // 2, TB], BF16, name=f"ago2_{hp}",
                              addr_space="Shared") for hp in range(2)]

            pending_vraw = {}

            def proj_steps(b, qb, preload=None):
                """Generator emitting the q/k/v projection + RoPE for block
                (b, qb); yields between instruction groups so attn_block can
                interleave the emission into its own stream (keeps the PE
                queue fed during ACT-paced stretches)."""
                tb = b * QBS + qb
                gs = slice(tb * TB, (tb + 1) * TB)      # global token slice
                ls = slice(qb * TB, (qb + 1) * TB)      # slice within batch
                h = hstream.tile([128, HCH, TB], BF16, tag="h")
                # 4 sliced DMAs so the first matmuls start after 1/4 of h
                for hc in range(4):
                    nc.sync.dma_start(h[:, 4 * hc:4 * hc + 4, :],
                                      hT[:, tb, 4 * hc:4 * hc + 4, :])
                if preload is not None:
                    preload()
                yield
                # pass A: q heads 0,1 + kv
                pqA = psM.tile([128, TB], F32, tag="A", name=f"pqA_{tb}")
                pkv = psM.tile([128, TB], F32, tag="Bk", name=f"pkv_{tb}")
                for cc in range(HCH):
                    nc.tensor.matmul(pqA[:], wq_sb[:, cc, 0:128], h[:, cc, :],
                                     start=(cc == 0), stop=(cc == HCH - 1))
                    nc.tensor.matmul(pkv[:], wkv_sb[:, cc, :], h[:, cc, :],
                                     start=(cc == 0), stop=(cc == HCH - 1))
                    if cc % 2 == 1:
                        yield
                # evacuate pkv: V rows 0:64, K rows 64:128
                kraw = rope.tile([DH, TB], BF16, tag="kraw")
                nc.vector.tensor_copy(kraw[:], pkv[64:128, :])
                vraw = rope.tile([DH, TB], BF16, tag="vraw")
                nc.vector.tensor_copy(vraw[:], pkv[0:64, :])
                yield
                # evacuate q pair A early (frees psM tag A for the K rope)
                qrawA = rope.tile([128, TB], BF16, tag="qrawA")
                nc.vector.tensor_copy(qrawA[:], pqA[:])
                yield

                # rotate + combine, two heads per unit
                unit_idx = [0]

                def rope_unit(raw, dst, np_):
                    """raw/dst: [np_, TB] bf16 (np_=128 pair, 64 for K)."""
                    qcos = rope.tile([np_, TB], BF16, tag="cos")
                    nc.vector.tensor_tensor(qcos[:], raw[:], cos_sb[0:np_, gs],
                                            MUL)
                    # rot-MM lands in the proj banks (freed by the evacs)
                    tag = "A" if unit_idx[0] % 2 == 0 else "Bk"
                    unit_idx[0] += 1
                    rps = psM.tile([128, TB], F32, tag=tag,
                                   name=f"rps_{tb}_{id(raw)}")
                    nc.tensor.matmul(rps[0:np_, :], rot2_sb[0:np_, 0:np_],
                                     raw[:], start=True, stop=True)
                    yield
                    rot_bf = rope.tile([np_, TB], BF16, tag="rotb")
                    nc.vector.tensor_copy(rot_bf[:], rps[0:np_, :])
                    qsin = rope.tile([np_, TB], BF16, tag="sin")
                    nc.vector.tensor_tensor(qsin[:], rot_bf[:],
                                            sin_sb[0:np_, gs], MUL)
                    nc.vector.tensor_tensor(dst, qcos[:], qsin[:], ADD)
                    yield

                # K rope FIRST (tag A, freed by qrawA evac) + row-dup DMA:
                # early emission so the dup lands well before the next
                # block's x=1 score matmuls need rows 64:128
                yield from rope_unit(kraw, kT_sb[b][0:64, ls], DH)
                nc.sync.dma_start(kT_sb[b][64:128, ls], kT_sb[b][0:64, ls])
                yield
                # pass B: q heads 2,3 (tag Bk, freed by kraw/vraw evacs)
                pqB = psM.tile([128, TB], F32, tag="Bk", name=f"pqB_{tb}")
                for cc in range(HCH):
                    nc.tensor.matmul(pqB[:], wq_sb[:, cc, 128:256], h[:, cc, :],
                                     start=(cc == 0), stop=(cc == HCH - 1))
                    if cc % 2 == 1:
                        yield
                # V transpose deferred to emit_vtrans (PE-based; DMA-transpose
                # serializes against in-flight collectives and stalls Sync)
                pending_vraw[(b, qb)] = vraw
                qrawB = rope.tile([128, TB], BF16, tag="qrawB")
                nc.vector.tensor_copy(qrawB[:], pqB[:])
                yield
                yield from rope_unit(qrawA, qT_sb[b][0][:, ls], 128)
                yield from rope_unit(qrawB, qT_sb[b][1][:, ls], 128)

            def emit_vtrans(b, qb):
                """Transpose vraw [64, TB] -> v_sb [128tok, 4, 64d] via 4 PE
                matmuls against identity (vraw_chunk.T @ I), borrowing a
                score-pool PSUM buffer; evacuated by one strided DVE cast.
                Emitted at block boundaries (collective-free windows)."""
                vraw = pending_vraw.pop((b, qb))
                vt = psS.tile([128, 2, TB], F32, tag="s",
                              name=f"vt_{b}_{qb}")
                for i in range(TB // KC):
                    nc.tensor.matmul(vt[:, 0, i * DH:(i + 1) * DH],
                                     vraw[:, i * KC:(i + 1) * KC],
                                     id64_sb[:], start=True, stop=True)
                for i in range(TB // KC):
                    nc.vector.tensor_copy(
                        v_sb[b][:, qb * (TB // KC) + i, 0:DH],
                        vt[:, 0, i * DH:(i + 1) * DH])

            def oproj_tasks(b, qb):
                """Return list of thunks computing outT for (b, qb)."""
                gs = slice((b * QBS + qb) * TB, (b * QBS + qb + 1) * TB)
                tasks = []
                state = {}
                c = cstream.tile([128, HCH, TB], BF16, tag="c",
                                 name=f"c_{b}_{qb}")
                # 4 chunked loads (cheap issue, lets the first matmuls start
                # after a quarter); hoisted to the feed front by the caller
                for q4 in range(4):
                    tasks.append(lambda q4=q4: nc.sync.dma_start(
                        c[:, 4 * q4:4 * q4 + 4, :],
                        ago[b][qb][512 * q4:512 * (q4 + 1), :].rearrange(
                            "(o p) m -> p o m", p=128)))

                def mk_po(o):
                    state[o] = psM.tile([128, TB], F32,
                                        tag=("A" if o == 0 else "Bk"),
                                        name=f"po{o}_{b}_{qb}")

                def mm(o, cc, first, last):
                    nc.tensor.matmul(state[o][:],
                                     wo_sb[:, cc, o * 128:(o + 1) * 128],
                                     c[:, cc, :], start=first, stop=last)

                def fin(o):
                    osb = attn.tile([128, TB], F32, tag="osb")
                    nc.vector.tensor_copy(osb[:], state[o][:])
                    nc.sync.dma_start(outT[o * 128:(o + 1) * 128, gs], osb[:])

                for o in range(2):
                    tasks.append(lambda o=o: mk_po(o))
                    for j, cc in enumerate(range(HCH)):
                        tasks.append(lambda o=o, cc=cc, f=(j == 0),
                                     l=(j == HCH - 1): mm(o, cc, f, l))
                    tasks.append(lambda o=o: fin(o))
                return tasks

            def oproj_split_tasks(b, qb):
                """Last block's o_proj as (phaseA, phaseB): phaseA consumes
                the hp0 half-AllGather and drains during hp1's attention;
                phaseB (hp1 half + stores) is the only tail work."""
                gs = slice((b * QBS + qb) * TB, (b * QBS + qb + 1) * TB)
                state = {}
                # c[p, hp, o, m] = ago2[hp][128*o + p, m]; wo chunk cc
                # (global ctx rows [128cc,128cc+128)) maps to hp=cc%2, o=cc//2
                c = cstream.tile([128, 2, HCH // 2, TB], BF16, tag="c",
                                 name=f"c_{b}_{qb}")

                def load(hp):
                    nc.sync.dma_start(
                        c[:, hp, :, :],
                        ago2[hp][:].rearrange("(o p) m -> p o m", p=128))

                def mk_po(o):
                    state[o] = psM.tile([128, TB], F32,
                                        tag=("A" if o == 0 else "Bk"),
                                        name=f"po{o}_{b}_{qb}")

                def mm(o, cc, first, last):
                    nc.tensor.matmul(state[o][:],
                                     wo_sb[:, cc, o * 128:(o + 1) * 128],
                                     c[:, cc % 2, cc // 2, :],
                                     start=first, stop=last)

                def fin(o):
                    osb = attn.tile([128, TB], F32, tag="osb")
                    nc.vector.tensor_copy(osb[:], state[o][:])
                    nc.sync.dma_start(outT[o * 128:(o + 1) * 128, gs], osb[:])

                pa = [lambda: load(0), lambda: mk_po(0), lambda: mk_po(1)]
                for cc in range(0, HCH, 2):
                    for o in range(2):
                        pa.append(lambda o=o, cc=cc, f=(cc == 0):
                                  mm(o, cc, f, False))
                pb = [lambda: load(1)]
                for cc in range(1, HCH, 2):
                    for o in range(2):
                        pb.append(lambda o=o, cc=cc, l=(cc == HCH - 1):
                                  mm(o, cc, False, l))
                pb += [lambda: fin(0), lambda: fin(1)]
                return pa, pb

            def attn_block(b, qb, feed, split_ag=False, post_hp=None):
                def drain(n):
                    for _ in range(n):
                        if not feed.step():
                            return

                for hp in range(2):
                    ctxp = psC.tile([DH + 1, 2, TB], F32, tag="x",
                                    name=f"ctx_{b}_{qb}_{hp}")
                    kcs = list(range(4 * qb + 4))
                    for i, kc in enumerate(kcs):
                        c0 = 128 * (kc - 4 * qb) if kc >= 4 * qb else 0
                        nsl = slice(c0, TB)
                        qsl = slice(qb * TB + c0, (qb + 1) * TB)
                        sps = psS.tile([128, 2, TB], F32, tag="s",
                                       name=f"s_{b}_{qb}_{hp}_{kc}")
                        for x, off in ((0, 0), (1, 64)):
                            nc.tensor.matmul(
                                sps[:, x, nsl],
                                kT_sb[b][off:off + 64, kc * KC:(kc + 1) * KC],
                                qT_sb[b][hp][off:off + 64, qsl],
                                start=True, stop=True)
                        if kc >= 4 * qb:
                            for x in range(2):
                                nc.vector.tensor_tensor(
                                    sps[:, x, c0:c0 + 128],
                                    sps[:, x, c0:c0 + 128], mk_sb[:], ADD)
                        p = attn.tile([128, 2, TB], BF16, tag="p")
                        if EXP_MERGE:
                            nc.scalar.activation(p[:, :, nsl], sps[:, :, nsl],
                                                 EXPF, scale=SCALE)
                        else:
                            for x in range(2):
                                nc.scalar.activation(p[:, x, nsl],
                                                     sps[:, x, nsl],
                                                     EXPF, scale=SCALE)
                        for x in range(2):
                            nc.tensor.matmul(ctxp[:, x, nsl],
                                             v_sb[b][:, kc, 0:DH + 1],
                                             p[:, x, nsl], start=(i == 0),
                                             stop=(i == len(kcs) - 1),
                                             skip_group_check=True)
                        drain(2)
                    # normalize both heads of the pair; pack into one tile
                    ctxn = attn.tile([DH, 2, TB], BF16, tag="ctxn")
                    for x in range(2):
                        # fast approx reciprocal (~18 bits, 5x faster than
                        # DVE reciprocal; denom > 0 always so no edge cases).
                        # Stage the denominator row to SBUF via the Scalar
                        # engine (idle at block ends): the custom-DVE op
                        # misreads partition-64 PSUM sources directly.
                        rc = attn.tile([1, TB], F32, tag="rc")
                        dcp = attn.tile([1, TB], F32, tag="dcp")
                        nc.scalar.copy(dcp[:], ctxp[DH:DH + 1, x, :])
                        nc.vector.reciprocal_approx_fast(rc[:], dcp[:])
                        rb = attn.tile([DH, TB], F32, tag="rb")
                        nc.gpsimd.partition_broadcast(rb[:], rc[:])
                        nc.vector.tensor_tensor(ctxn[:, x, :],
                                                ctxp[0:DH, x, :], rb[:], MUL)
                        drain(2)
                    # two plain contiguous DMAs: the interleaved-rearrange
                    # single DMA cost 8.3us of Sync issue time
                    for x in range(2):
                        if split_ag:
                            nc.sync.dma_start(agi2[hp][64 * x:64 * x + 64, :],
                                              ctxn[:, x, :])
                        else:
                            r0 = 128 * hp + 64 * x
                            nc.sync.dma_start(agi[b][qb][r0:r0 + 64, :],
                                              ctxn[:, x, :])
                    if split_ag:
                        nc.gpsimd.collective_compute(
                            "AllGather", mybir.AluOpType.bypass,
                            replica_groups=[list(range(NC))],
                            ins=[agi2[hp].opt()], outs=[ago2[hp].opt()])
                    if post_hp is not None:
                        post_hp(hp)
                    drain(2)

            import concourse.mybir as _mybir

            class Feed:
                """Drain source for attn_block: first advances the next
                block's proj emission (so its PSUM-bank claims stay ahead of
                o_proj's), then pops queued o_proj tasks."""

                def __init__(self, gen, tasks):
                    self.gen = gen
                    self.tasks = tasks

                def step(self):
                    if self.gen is not None:
                        try:
                            next(self.gen)
                            return True
                        except StopIteration:
                            self.gen = None
                    if self.tasks:
                        self.tasks.pop(0)()
                        return True
                    return False

            # software pipeline: proj(n+1) emission is interleaved INTO
            # attn(n)'s drain slots (ahead of o_proj tasks), so the PE queue
            # always holds ready work while attn(n) waits on ACT.
            # o_proj(n) drains during attn(n+3) mid-stream; the last two
            # attention blocks each drain two o_projs so only the final
            # (split-AG) o_proj remains in the tail.
            blocks = [(b, qb) for b in range(B) for qb in range(QBS)]
            queue = []             # FIFO of oproj task lists
            for _ in proj_steps(*blocks[0], preload=load_consts):
                pass
            emit_vtrans(*blocks[0])
            load_wo()
            for i, (b, qb) in enumerate(blocks):
                gen = (proj_steps(*blocks[i + 1])
                       if i + 1 < len(blocks) else None)
                last = (i == len(blocks) - 1)
                npop = (2 if i >= 6 else (1 if i >= 3 else 0))
                loads, tasks = [], []
                while queue and npop > 0:
                    lst = queue.pop(0)
                    loads += lst[:4]     # hoist the 4 chunked c-loads
                    tasks += lst[4:]
                    npop -= 1
                tasks = loads + tasks
                feed = Feed(gen, tasks)
                if last:
                    pa, pb = oproj_split_tasks(b, qb)

                    def post_hp(hp):
                        if hp == 0:
                            feed.tasks += pa
                        else:
                            # issue the hp1 c-load now: the DMA parks on the
                            # AllGather semaphore and fires the moment the
                            # gathered data lands
                            pb.pop(0)()
                    attn_block(b, qb, feed, split_ag=True, post_hp=post_hp)
                else:
                    attn_block(b, qb, feed)
                # finish proj(n+1) emission (incl. its PE V-transpose)
                # BEFORE the AllGather so nothing serializes behind it
                if feed.gen is not None:
                    for _ in feed.gen:
                        pass
                    feed.gen = None
                if i + 1 < len(blocks):
                    emit_vtrans(*blocks[i + 1])
                if not last:
                    nc.gpsimd.collective_compute(
                        "AllGather", _mybir.AluOpType.bypass,
                        replica_groups=[list(range(NC))],
                        ins=[agi[b][qb].opt()], outs=[ago[b][qb].opt()])
                while feed.step():
                    pass
                if not last:
                    queue.append(oproj_tasks(b, qb))
                else:
                    for t in pb:
                        t()

    nc.compile()
    return nc


def _build_general():
    """Fallback f32 path for non-causal masks (full mask streamed)."""
    import concourse.mybir as mybir
    import concourse.tile as tile
    from concourse import bacc
    from concourse.masks import make_identity

    F32 = mybir.dt.float32
    F32R = mybir.dt.float32r
    EXPF = mybir.ActivationFunctionType.Exp
    ADD = mybir.AluOpType.add
    MUL = mybir.AluOpType.mult

    nc = bacc.Bacc("TRN2", target_bir_lowering=False, debug=False, num_devices=NC)

    hT = nc.dram_tensor("hT", [HID, T], F32R, kind="ExternalInput")
    wqT = nc.dram_tensor("wqT", [HID, CPC], F32R, kind="ExternalInput")
    wkvT = nc.dram_tensor("wkvT", [HID, 2 * DH], F32R, kind="ExternalInput")
    woT = nc.dram_tensor("woT", [H * DH, CPC], F32R, kind="ExternalInput")
    cosT = nc.dram_tensor("cosT", [DH, T], F32, kind="ExternalInput")
    sinT = nc.dram_tensor("sinT", [DH, T], F32, kind="ExternalInput")
    rotp = nc.dram_tensor("rotp", [DH, DH], F32R, kind="ExternalInput")
    maskg = nc.dram_tensor("maskg", [S, S], F32, kind="ExternalInput")
    outT = nc.dram_tensor("outT", [CPC, T], F32, kind="ExternalOutput")

    with tile.TileContext(nc) as tc:
        with tc.tile_pool(name="const", bufs=1) as cpool, \
             tc.tile_pool(name="big", bufs=1) as big, \
             tc.tile_pool(name="stream", bufs=3) as stream, \
             tc.tile_pool(name="rope", bufs=2) as rope, \
             tc.tile_pool(name="attn", bufs=3) as attn, \
             tc.tile_pool(name="psM", bufs=1, space="PSUM") as psM, \
             tc.tile_pool(name="psS", bufs=3, space="PSUM") as psS, \
             tc.tile_pool(name="psC", bufs=1, space="PSUM") as psC, \
             tc.tile_pool(name="dram", bufs=1, space="DRAM") as dram:

            wq_sb = cpool.tile([128, HCH, CPC], F32R)
            nc.sync.dma_start(wq_sb[:], wqT[:].rearrange("(o p) m -> p o m", p=128))
            wkv_sb = cpool.tile([128, HCH, 2 * DH], F32R)
            nc.sync.dma_start(wkv_sb[:], wkvT[:].rearrange("(o p) m -> p o m", p=128))
            wo_sb = cpool.tile([128, HCH, CPC], F32R)
            nc.sync.dma_start(wo_sb[:], woT[:].rearrange("(o p) m -> p o m", p=128))
            cos_sb = cpool.tile([DH, T], F32)
            nc.sync.dma_start(cos_sb[:], cosT[:])
            sin_sb = cpool.tile([DH, T], F32)
            nc.sync.dma_start(sin_sb[:], sinT[:])
            rot_sb = cpool.tile([DH, DH], F32R)
            nc.sync.dma_start(rot_sb[:], rotp[:])
            onesc_f = cpool.tile([128, SB_KC, 1], F32)
            nc.any.memset(onesc_f[:], 1.0)
            ident = cpool.tile([DH, DH], F32)
            make_identity(nc, ident)

            qT_sb = [[big.tile([128, S], F32R, tag=f"qT{b}{hp}", name=f"qT{b}{hp}")
                      for hp in range(2)] for b in range(B)]
            kT_sb = [big.tile([128, S], F32R, tag=f"kT{b}", name=f"kT{b}")
                     for b in range(B)]
            v_sb = [big.tile([128, SB_KC, DH + 1], F32R, tag=f"v{b}", name=f"v{b}")
                    for b in range(B)]
            for b in range(B):
                nc.vector.tensor_copy(v_sb[b][:, :, DH:DH + 1], onesc_f[:])

            ag_in = [dram.tile([CPC, S], F32R, name=f"agi{b}") for b in range(B)]
            ag_out = [dram.tile([H * DH, S], F32R, name=f"ago{b}",
                                addr_space="Shared") for b in range(B)]

            def proj_block(b, qb):
                tb = b * QBS + qb
                gs = slice(tb * TB, (tb + 1) * TB)
                ls = slice(qb * TB, (qb + 1) * TB)
                pq = [psM.tile([128, TB], F32, tag=f"mm{hp}", name=f"pq{hp}_{tb}")
                      for hp in range(2)]
                pkv = psM.tile([128, TB], F32, tag="mmkv")
                for cc in range(HCH):
                    h_sb = stream.tile([128, TB], F32R, tag="h")
                    nc.sync.dma_start(h_sb[:], hT[cc * 128:(cc + 1) * 128, gs])
                    for hp in range(2):
                        nc.tensor.matmul(pq[hp][:],
                                         wq_sb[:, cc, hp * 128:(hp + 1) * 128],
                                         h_sb[:], start=(cc == 0),
                                         stop=(cc == HCH - 1))
                    nc.tensor.matmul(pkv[:], wkv_sb[:, cc, :], h_sb[:],
                                     start=(cc == 0), stop=(cc == HCH - 1))
                for h in range(HPC):
                    hp, hh = h // 2, 64 * (h % 2)
                    src = pq[hp][hh:hh + 64, :]
                    qraw = rope.tile([DH, TB], F32R, tag="raw")
                    nc.vector.tensor_copy(qraw[:], src)
                    qcos = rope.tile([DH, TB], F32, tag="cos")
                    nc.vector.tensor_tensor(qcos[:], qraw[:].bitcast(F32),
                                            cos_sb[:, gs], MUL)
                    rps = psS.tile([DH, TB], F32, tag="s")
                    nc.tensor.matmul(rps[:], rot_sb[:], qraw[:], start=True,
                                     stop=True)
                    qsin = rope.tile([DH, TB], F32, tag="sin")
                    nc.vector.tensor_tensor(qsin[:], rps[:], sin_sb[:, gs], MUL)
                    if hh == 0:
                        nc.vector.tensor_tensor(qT_sb[b][hp][0:64, ls],
                                                qcos[:], qsin[:], ADD)
                    else:
                        qfin = rope.tile([DH, TB], F32R, tag="fin")
                        nc.vector.tensor_tensor(qfin[:], qcos[:], qsin[:], ADD)
                        nc.sync.dma_start(qT_sb[b][hp][64:128, ls], qfin[:])
                ksrc = pkv[64:128, :]
                kraw = rope.tile([DH, TB], F32R, tag="raw")
                nc.vector.tensor_copy(kraw[:], ksrc)
                kcos = rope.tile([DH, TB], F32, tag="cos")
                nc.vector.tensor_tensor(kcos[:], kraw[:].bitcast(F32),
                                        cos_sb[:, gs], MUL)
                krps = psS.tile([DH, TB], F32, tag="s")
                nc.tensor.matmul(krps[:], rot_sb[:], kraw[:], start=True, stop=True)
                ksin = rope.tile([DH, TB], F32, tag="sin")
                nc.vector.tensor_tensor(ksin[:], krps[:], sin_sb[:, gs], MUL)
                nc.vector.tensor_tensor(kT_sb[b][0:64, ls], kcos[:], ksin[:], ADD)
                nc.sync.dma_start(kT_sb[b][64:128, ls], kT_sb[b][0:64, ls])
                vraw = rope.tile([DH, TB], F32, tag="vraw")
                nc.vector.tensor_copy(vraw[:], pkv[0:64, :])
                for i in range(TB // KC):
                    vtp = psS.tile([128, DH], F32, tag="s")
                    nc.tensor.transpose(vtp[:], vraw[:, i * KC:(i + 1) * KC],
                                        ident[:])
                    nc.vector.tensor_copy(v_sb[b][:, qb * (TB // KC) + i, 0:DH],
                                          vtp[:])

            def attn_block(b, qb):
                for hp in range(2):
                    kcs = list(range(SB_KC))
                    ctxp = [psC.tile([DH + 1, TB], F32, tag=f"ctx{x}",
                                     name=f"ctx{x}_{b}_{qb}_{hp}")
                            for x in range(2)]
                    for i, kc in enumerate(kcs):
                        sps = [psS.tile([128, TB], F32, tag="s",
                                        name=f"s{x}_{b}_{qb}_{hp}_{kc}")
                               for x in range(2)]
                        for x, hh in enumerate((0, 64)):
                            nc.tensor.matmul(
                                sps[x][:],
                                kT_sb[b][hh:hh + 64, kc * KC:(kc + 1) * KC],
                                qT_sb[b][hp][hh:hh + 64,
                                             qb * TB:(qb + 1) * TB],
                                start=True, stop=True)
                        for x in range(2):
                            mg = attn.tile([128, TB], F32, tag="mg")
                            nc.sync.dma_start(
                                mg[:], maskg[kc * KC:(kc + 1) * KC,
                                             qb * TB:(qb + 1) * TB])
                            nc.vector.tensor_tensor(sps[x][:], sps[x][:],
                                                    mg[:], ADD)
                            p_sb = attn.tile([128, TB], F32R, tag="p")
                            nc.scalar.activation(p_sb[:], sps[x][:],
                                                 EXPF, scale=SCALE)
                            nc.tensor.matmul(ctxp[x][:], v_sb[b][:, kc, :],
                                             p_sb[:], start=(i == 0),
                                             stop=(i == len(kcs) - 1),
                                             skip_group_check=True)
                    for x in range(2):
                        h = 2 * hp + x
                        rc = attn.tile([1, TB], F32R, tag="rc")
                        with nc.allow_low_precision(reason="f32r rounding ~1e-4"):
                            nc.vector.reciprocal(rc[:], ctxp[x][DH:DH + 1, :])
                        rb = attn.tile([DH, TB], F32R, tag="rb")
                        nc.gpsimd.partition_broadcast(rb[:], rc[:])
                        ctxn = attn.tile([DH, TB], F32R, tag="ctxn")
                        nc.vector.tensor_tensor(ctxn[:], ctxp[x][0:DH, :],
                                                rb[:].bitcast(F32), MUL)
                        nc.sync.dma_start(
                            ag_in[b][h * DH:(h + 1) * DH,
                                     qb * TB:(qb + 1) * TB],
                            ctxn[:])

            for b in range(B):
                for qb in range(QBS):
                    proj_block(b, qb)
                    attn_block(b, qb)
                import concourse.mybir as _mybir
                nc.gpsimd.collective_compute(
                    "AllGather", _mybir.AluOpType.bypass,
                    replica_groups=[list(range(NC))],
                    ins=[ag_in[b].opt()], outs=[ag_out[b].opt()])

            for b in range(B):
                for qb in range(QBS):
                    ls = slice(qb * TB, (qb + 1) * TB)
                    gs = slice((b * QBS + qb) * TB, (b * QBS + qb + 1) * TB)
                    po = [psM.tile([128, TB], F32, tag=f"mm{o}",
                                   name=f"po{o}_{b}_{qb}")
                          for o in range(2)]
                    for cc in range(HCH):
                        c_sb = stream.tile([128, TB], F32R, tag="c")
                        nc.sync.dma_start(
                            c_sb[:], ag_out[b][cc * 128:(cc + 1) * 128, ls])
                        for o in range(2):
                            nc.tensor.matmul(po[o][:],
                                             wo_sb[:, cc, o * 128:(o + 1) * 128],
                                             c_sb[:], start=(cc == 0),
                                             stop=(cc == HCH - 1))
                    for o in range(2):
                        o_sb = stream.tile([128, TB], F32, tag="o")
                        nc.vector.tensor_copy(o_sb[:], po[o][:])
                        nc.sync.dma_start(outT[o * 128:(o + 1) * 128, gs],
                                          o_sb[:])

    nc.compile()
    return nc


def _host_inputs(hidden_states, cos, sin, attention_mask, Wq, Wk, Wv, Wo,
                 causal):
    hT = np.ascontiguousarray(hidden_states.reshape(T, HID).T)
    cosT = np.ascontiguousarray(cos.reshape(T, DH).T)
    sinT = np.ascontiguousarray(sin.reshape(T, DH).T)
    # rot_half as a signed permutation: rot[d] = -x[d+32] (d<32), +x[d-32]
    p64 = np.zeros((DH, DH), np.float32)
    for m in range(32):
        p64[m + 32, m] = -1.0
        p64[m, m + 32] = 1.0
    WqT = np.ascontiguousarray(Wq.T)      # [HID, H*DH]
    WkT = np.ascontiguousarray(Wk.T)      # [HID, HKV*DH]
    WvT = np.ascontiguousarray(Wv.T)
    WoT = np.ascontiguousarray(Wo.T)      # [H*DH, HID]

    if causal:
        import ml_dtypes
        bf = ml_dtypes.bfloat16
        # partition-major pre-arrangements: X[o*128+p, m] -> X3[p, o, m]
        # (and per-block for hT) so each device DMA is one contiguous run
        # per partition
        hT = np.ascontiguousarray(
            hT.astype(bf).reshape(HCH, 128, B * S // 512, 512)
            .transpose(1, 2, 0, 3))
        WqT, WkT, WvT, WoT = (w.astype(bf) for w in (WqT, WkT, WvT, WoT))
        cosT = cosT.astype(bf)
        sinT = sinT.astype(bf)
        # block-diag rot for head-paired RoPE ([128,128]); upper-left 64x64
        # block doubles as the single-head (K) rot matrix
        p128 = np.zeros((128, 128), np.float32)
        p128[0:DH, 0:DH] = p64
        p128[DH:128, DH:128] = p64
        p128 = p128.astype(bf)
    def pmaj(w):
        """[HCH*128, m] -> [128, HCH, m] partition-major (causal path)."""
        if not causal:
            return np.ascontiguousarray(w)
        return np.ascontiguousarray(
            w.reshape(HCH, 128, w.shape[1]).transpose(1, 0, 2))

    ins = []
    for c in range(NC):
        d = {
            "hT": hT,
            "wqT": pmaj(WqT[:, c * CPC:(c + 1) * CPC]),
            "wkvT": pmaj(
                np.concatenate([WvT[:, c * DH:(c + 1) * DH],
                                WkT[:, c * DH:(c + 1) * DH]], axis=1)),
            "woT": pmaj(WoT[:, c * CPC:(c + 1) * CPC]),
            "cosT": cosT, "sinT": sinT,
        }
        if causal:
            d["rot2p"] = p128
            d["id64"] = np.eye(DH, dtype=np.float32).astype(p128.dtype)
        else:
            d["rotp"] = p64
        if causal:
            i = np.arange(128, dtype=np.float32)[:, None]
            cc = np.arange(128, dtype=np.float32)[None, :]
            d["maskd"] = np.where(cc < i, NEG, 0.0).astype(np.float32)
        else:
            m = attention_mask[0, 0].astype(np.float32)
            d["maskg"] = np.ascontiguousarray(m.T) * np.float32(1.0 / SCALE)
        ins.append(d)
    return ins


def _is_causal(attention_mask):
    if attention_mask.shape != (1, 1, S, S):
        return False
    m = attention_mask[0, 0]
    neg = np.finfo(np.float32).min
    tril = np.tril(np.ones((S, S), dtype=bool))
    expect = np.where(tril, np.float32(0.0), np.float32(neg))
    return np.array_equal(m, expect)


_CACHE = {}


def _get_nc(causal):
    if causal not in _CACHE:
        _CACHE[causal] = _build_fast() if causal else _build_general()
    return _CACHE[causal]


def kernel(**inputs) -> np.ndarray:
    from concourse.bass_utils import run_bass_kernel_spmd

    hidden_states = np.asarray(inputs["hidden_states"], np.float32)
    cos = np.asarray(inputs["cos"], np.float32)
    sin = np.asarray(inputs["sin"], np.float32)
    attention_mask = np.asarray(inputs["attention_mask"], np.float32)
    Wq = np.asarray(inputs["Wq"], np.float32)
    Wk = np.asarray(inputs["Wk"], np.float32)
    Wv = np.asarray(inputs["Wv"], np.float32)
    Wo = np.asarray(inputs["Wo"], np.float32)

    causal = _is_causal(attention_mask)
    nc = _get_nc(causal)
    ins = _host_inputs(hidden_states, cos, sin, attention_mask,
                       Wq, Wk, Wv, Wo, causal)
    res = run_bass_kernel_spmd(nc, ins, core_ids=list(range(NC)))
    outT = np.concatenate([res.results[c]["outT"] for c in range(NC)], axis=0)
    return np.ascontiguousarray(outT.T).reshape(B, S, HID)



# revision 56
# speedup vs baseline: 1.0196x; 1.0008x over previous
"""Llama GQA attention (B=2, S=2048, HID=2048, H=32, HKV=8, DH=64) on 8 TRN2 cores.

Sharding: tensor-parallel over heads. Core c owns q heads [4c, 4c+4) and kv
head c. One SPMD NEFF per run.

Fast causal path (bf16):
  1. Q/K/V projections in transposed layout with bf16 operands (full PE
     rate; fp32 matmuls run at half rate in fp32_mode=HIGH),
  2. RoPE via a signed-permutation matmul + DVE combines (bf16 in, f32
     combine, bf16 out),
  3. causal flash attention with scores kept transposed [k, q]; the two
     heads of a pair compute scores concurrently in PE row-groups 0/64 and
     share one [128, 2, 512] PSUM tile so a single ACT instruction
     exponentiates both (the ACT engine is the attention bottleneck:
     (N+352)-cycle cost per instruction, so batching the free dim matters),
  4. per-(batch, 512-token block) chunked AllGather of the normalized
     context (bf16), issued as soon as the block's attention finishes so
     collectives overlap compute,
  5. column-sharded o_proj per block, emitted as micro-tasks interleaved
     into the NEXT attention block's instruction stream so its matmuls
     fill PE bubbles while ACT works through the exps.
Host pre-transposes inputs (bf16) and assembles the 8 output slices.
"""
import sys

sys.path.insert(0, "/opt/trn_rl_repo")

import numpy as np

B, S, HID = 2, 2048, 2048
H, HKV, DH = 32, 8, 64
NC = 8
T = B * S
HPC = H // NC            # q heads per core (4)
CPC = HPC * DH           # ctx dims per core (256)
TB = 512                 # token block
KC = 128                 # k chunk
QBS = S // TB            # 4 q blocks per batch
SB_KC = S // KC          # 16 k chunks per batch
HCH = HID // 128         # 16 hid chunks
SCALE = DH ** -0.5
NEG = -1.0e30
EXP_MERGE = True


def _build_fast():
    """Causal, bf16, chunked-AG, interleaved o_proj."""
    import concourse.mybir as mybir
    import concourse.tile as tile
    from concourse import bacc
    F32 = mybir.dt.float32
    BF16 = mybir.dt.bfloat16
    EXPF = mybir.ActivationFunctionType.Exp
    ADD = mybir.AluOpType.add
    MUL = mybir.AluOpType.mult

    nc = bacc.Bacc("TRN2", target_bir_lowering=False, debug=False, num_devices=NC)

    # host pre-arranged to partition-major layouts: one contiguous run per
    # partition per DMA (descriptor-issue time on Sync is ~10x cheaper than
    # the "(o p) m -> p o m" rearrange form)
    hT = nc.dram_tensor("hT", [128, B * QBS, HCH, TB], BF16,
                        kind="ExternalInput")
    wqT = nc.dram_tensor("wqT", [128, HCH, CPC], BF16, kind="ExternalInput")
    wkvT = nc.dram_tensor("wkvT", [128, HCH, 2 * DH], BF16,
                          kind="ExternalInput")
    woT = nc.dram_tensor("woT", [128, HCH, CPC], BF16, kind="ExternalInput")
    cosT = nc.dram_tensor("cosT", [DH, T], BF16, kind="ExternalInput")
    sinT = nc.dram_tensor("sinT", [DH, T], BF16, kind="ExternalInput")
    rot2p = nc.dram_tensor("rot2p", [128, 128], BF16, kind="ExternalInput")
    id64 = nc.dram_tensor("id64", [DH, DH], BF16, kind="ExternalInput")
    maskd = nc.dram_tensor("maskd", [128, 128], F32, kind="ExternalInput")
    outT = nc.dram_tensor("outT", [CPC, T], F32, kind="ExternalOutput")

    with tile.TileContext(nc) as tc:
        with tc.tile_pool(name="const", bufs=1) as cpool, \
             tc.tile_pool(name="big", bufs=1) as big, \
             tc.tile_pool(name="hstream", bufs=2) as hstream, \
             tc.tile_pool(name="cstream", bufs=2) as cstream, \
             tc.tile_pool(name="rope", bufs=3) as rope, \
             tc.tile_pool(name="attn", bufs=4) as attn, \
             tc.tile_pool(name="psM", bufs=1, space="PSUM") as psM, \
             tc.tile_pool(name="psS", bufs=2, space="PSUM") as psS, \
             tc.tile_pool(name="psC", bufs=1, space="PSUM") as psC, \
             tc.tile_pool(name="dram", bufs=1, space="DRAM") as dram:

            # ---- persistent SBUF (only wq/wkv loaded before first h) ----
            wq_sb = cpool.tile([128, HCH, CPC], BF16)
            nc.sync.dma_start(wq_sb[:, 0:HCH // 2, :], wqT[:, 0:HCH // 2, :])
            wkv_sb = cpool.tile([128, HCH, 2 * DH], BF16)
            nc.sync.dma_start(wkv_sb[:], wkvT[:])
            nc.sync.dma_start(wq_sb[:, HCH // 2:, :], wqT[:, HCH // 2:, :])
            # cos/sin duplicated across both 64-partition halves so one DVE
            # op covers a head PAIR ([128, TB] instead of 2x [64, TB])
            cos_sb = cpool.tile([128, T], BF16)
            sin_sb = cpool.tile([128, T], BF16)
            rot2_sb = cpool.tile([128, 128], BF16)
            id64_sb = cpool.tile([DH, DH], BF16)
            mk_sb = cpool.tile([128, 128], F32)
            wo_sb = cpool.tile([128, HCH, CPC], BF16)

            def load_consts():
                nc.sync.dma_start(rot2_sb[:], rot2p[:])
                nc.sync.dma_start(mk_sb[:], maskd[:])
                nc.sync.dma_start(id64_sb[:], id64[:])
                nc.sync.dma_start(cos_sb[0:DH, :], cosT[:])
                nc.sync.dma_start(cos_sb[DH:128, :], cosT[:])
                nc.sync.dma_start(sin_sb[0:DH, :], sinT[:])
                nc.sync.dma_start(sin_sb[DH:128, :], sinT[:])

            def load_wo():
                nc.sync.dma_start(wo_sb[:], woT[:])

            # ---- per-batch big activation buffers ----
            qT_sb = [[big.tile([128, S], BF16, tag=f"qT{b}{hp}", name=f"qT{b}{hp}")
                      for hp in range(2)] for b in range(B)]
            kT_sb = [big.tile([128, S], BF16, tag=f"kT{b}", name=f"kT{b}")
                     for b in range(B)]
            # 80-elem stride keeps each chunk 32B-aligned for DMA-transpose
            v_sb = [big.tile([128, SB_KC, 80], BF16, tag=f"v{b}", name=f"v{b}")
                    for b in range(B)]
            for b in range(B):
                nc.any.memset(v_sb[b][:, :, DH:DH + 1], 1.0)

            agi = [[dram.tile([CPC, TB], BF16, name=f"agi{b}_{qb}")
                    for qb in range(QBS)] for b in range(B)]
            ago = [[dram.tile([H * DH, TB], BF16, name=f"ago{b}_{qb}",
                              addr_space="Shared") for qb in range(QBS)]
                   for b in range(B)]
            # last block: per-hp split collective so hp0's AG overlaps hp1's
            # attention and only a half-size AG sits in the tail
            agi2 = [dram.tile([128, TB], BF16, name=f"agi2_{hp}")
                    for hp in range(2)]
            ago2 = [dram.tile([H * DH // 2, TB], BF16, name=f"ago2_{hp}",
                              addr_space="Shared") for hp in range(2)]

            pending_vraw = {}

            def proj_steps(b, qb, preload=None):
                """Generator emitting the q/k/v projection + RoPE for block
                (b, qb); yields between instruction groups so attn_block can
                interleave the emission into its own stream (keeps the PE
                queue fed during ACT-paced stretches)."""
                tb = b * QBS + qb
                gs = slice(tb * TB, (tb + 1) * TB)      # global token slice
                ls = slice(qb * TB, (qb + 1) * TB)      # slice within batch
                h = hstream.tile([128, HCH, TB], BF16, tag="h")
                # 4 sliced DMAs so the first matmuls start after 1/4 of h
                for hc in range(4):
                    nc.sync.dma_start(h[:, 4 * hc:4 * hc + 4, :],
                                      hT[:, tb, 4 * hc:4 * hc + 4, :])
                if preload is not None:
                    preload()
                yield
                # pass A: q heads 0,1 + kv
                pqA = psM.tile([128, TB], F32, tag="A", name=f"pqA_{tb}")
                pkv = psM.tile([128, TB], F32, tag="Bk", name=f"pkv_{tb}")
                for cc in range(HCH):
                    nc.tensor.matmul(pqA[:], wq_sb[:, cc, 0:128], h[:, cc, :],
                                     start=(cc == 0), stop=(cc == HCH - 1))
                    nc.tensor.matmul(pkv[:], wkv_sb[:, cc, :], h[:, cc, :],
                                     start=(cc == 0), stop=(cc == HCH - 1))
                    if cc % 2 == 1:
                        yield
                # evacuate pkv: V rows 0:64, K rows 64:128
                kraw = rope.tile([DH, TB], BF16, tag="kraw")
                nc.vector.tensor_copy(kraw[:], pkv[64:128, :])
                vraw = rope.tile([DH, TB], BF16, tag="vraw")
                nc.vector.tensor_copy(vraw[:], pkv[0:64, :])
                yield
                # evacuate q pair A early (frees psM tag A for the K rope)
                qrawA = rope.tile([128, TB], BF16, tag="qrawA")
                nc.vector.tensor_copy(qrawA[:], pqA[:])
                yield

                # rotate + combine, two heads per unit
                unit_idx = [0]

                def rope_unit(raw, dst, np_):
                    """raw/dst: [np_, TB] bf16 (np_=128 pair, 64 for K)."""
                    qcos = rope.tile([np_, TB], BF16, tag="cos")
                    nc.vector.tensor_tensor(qcos[:], raw[:], cos_sb[0:np_, gs],
                                            MUL)
                    # rot-MM lands in the proj banks (freed by the evacs)
                    tag = "A" if unit_idx[0] % 2 == 0 else "Bk"
                    unit_idx[0] += 1
                    rps = psM.tile([128, TB], F32, tag=tag,
                                   name=f"rps_{tb}_{id(raw)}")
                    nc.tensor.matmul(rps[0:np_, :], rot2_sb[0:np_, 0:np_],
                                     raw[:], start=True, stop=True)
                    yield
                    rot_bf = rope.tile([np_, TB], BF16, tag="rotb")
                    nc.vector.tensor_copy(rot_bf[:], rps[0:np_, :])
                    qsin = rope.tile([np_, TB], BF16, tag="sin")
                    nc.vector.tensor_tensor(qsin[:], rot_bf[:],
                                            sin_sb[0:np_, gs], MUL)
                    nc.vector.tensor_tensor(dst, qcos[:], qsin[:], ADD)
                    yield

                # K rope FIRST (tag A, freed by qrawA evac) + row-dup DMA:
                # early emission so the dup lands well before the next
                # block's x=1 score matmuls need rows 64:128
                yield from rope_unit(kraw, kT_sb[b][0:64, ls], DH)
                nc.sync.dma_start(kT_sb[b][64:128, ls], kT_sb[b][0:64, ls])
                yield
                # pass B: q heads 2,3 (tag Bk, freed by kraw/vraw evacs)
                pqB = psM.tile([128, TB], F32, tag="Bk", name=f"pqB_{tb}")
                for cc in range(HCH):
                    nc.tensor.matmul(pqB[:], wq_sb[:, cc, 128:256], h[:, cc, :],
                                     start=(cc == 0), stop=(cc == HCH - 1))
                    if cc % 2 == 1:
                        yield
                # V transpose deferred to emit_vtrans (PE-based; DMA-transpose
                # serializes against in-flight collectives and stalls Sync)
                pending_vraw[(b, qb)] = vraw
                qrawB = rope.tile([128, TB], BF16, tag="qrawB")
                nc.vector.tensor_copy(qrawB[:], pqB[:])
                yield
                yield from rope_unit(qrawA, qT_sb[b][0][:, ls], 128)
                yield from rope_unit(qrawB, qT_sb[b][1][:, ls], 128)

            def emit_vtrans(b, qb):
                """Transpose vraw [64, TB] -> v_sb [128tok, 4, 64d] via 4 PE
                matmuls against identity (vraw_chunk.T @ I), borrowing a
                score-pool PSUM buffer; evacuated by one strided DVE cast.
                Emitted at block boundaries (collective-free windows)."""
                vraw = pending_vraw.pop((b, qb))
                vt = psS.tile([128, 2, TB], F32, tag="s",
                              name=f"vt_{b}_{qb}")
                for i in range(TB // KC):
                    nc.tensor.matmul(vt[:, 0, i * DH:(i + 1) * DH],
                                     vraw[:, i * KC:(i + 1) * KC],
                                     id64_sb[:], start=True, stop=True)
                for i in range(TB // KC):
                    nc.vector.tensor_copy(
                        v_sb[b][:, qb * (TB // KC) + i, 0:DH],
                        vt[:, 0, i * DH:(i + 1) * DH])

            def oproj_tasks(b, qb):
                """Return list of thunks computing outT for (b, qb)."""
                gs = slice((b * QBS + qb) * TB, (b * QBS + qb + 1) * TB)
                tasks = []
                state = {}
                c = cstream.tile([128, HCH, TB], BF16, tag="c",
                                 name=f"c_{b}_{qb}")
                # 4 chunked loads (cheap issue, lets the first matmuls start
                # after a quarter); hoisted to the feed front by the caller
                for q4 in range(4):
                    tasks.append(lambda q4=q4: nc.sync.dma_start(
                        c[:, 4 * q4:4 * q4 + 4, :],
                        ago[b][qb][512 * q4:512 * (q4 + 1), :].rearrange(
                            "(o p) m -> p o m", p=128)))

                def mk_po(o):
                    state[o] = psM.tile([128, TB], F32,
                                        tag=("A" if o == 0 else "Bk"),
                                        name=f"po{o}_{b}_{qb}")

                def mm(o, cc, first, last):
                    nc.tensor.matmul(state[o][:],
                                     wo_sb[:, cc, o * 128:(o + 1) * 128],
                                     c[:, cc, :], start=first, stop=last)

                def fin(o):
                    osb = attn.tile([128, TB], F32, tag="osb")
                    nc.vector.tensor_copy(osb[:], state[o][:])
                    nc.sync.dma_start(outT[o * 128:(o + 1) * 128, gs], osb[:])

                for o in range(2):
                    tasks.append(lambda o=o: mk_po(o))
                    for j, cc in enumerate(range(HCH)):
                        tasks.append(lambda o=o, cc=cc, f=(j == 0),
                                     l=(j == HCH - 1): mm(o, cc, f, l))
                    tasks.append(lambda o=o: fin(o))
                return tasks

            def oproj_split_tasks(b, qb):
                """Last block's o_proj as (phaseA, phaseB): phaseA consumes
                the hp0 half-AllGather and drains during hp1's attention;
                phaseB (hp1 half + stores) is the only tail work."""
                gs = slice((b * QBS + qb) * TB, (b * QBS + qb + 1) * TB)
                state = {}
                # c[p, hp, o, m] = ago2[hp][128*o + p, m]; wo chunk cc
                # (global ctx rows [128cc,128cc+128)) maps to hp=cc%2, o=cc//2
                c = cstream.tile([128, 2, HCH // 2, TB], BF16, tag="c",
                                 name=f"c_{b}_{qb}")

                def load(hp):
                    nc.sync.dma_start(
                        c[:, hp, :, :],
                        ago2[hp][:].rearrange("(o p) m -> p o m", p=128))

                def mk_po(o):
                    state[o] = psM.tile([128, TB], F32,
                                        tag=("A" if o == 0 else "Bk"),
                                        name=f"po{o}_{b}_{qb}")

                def mm(o, cc, first, last):
                    nc.tensor.matmul(state[o][:],
                                     wo_sb[:, cc, o * 128:(o + 1) * 128],
                                     c[:, cc % 2, cc // 2, :],
                                     start=first, stop=last)

                def fin(o):
                    osb = attn.tile([128, TB], F32, tag="osb")
                    nc.vector.tensor_copy(osb[:], state[o][:])
                    nc.sync.dma_start(outT[o * 128:(o + 1) * 128, gs], osb[:])

                pa = [lambda: load(0), lambda: mk_po(0), lambda: mk_po(1)]
                for cc in range(0, HCH, 2):
                    for o in range(2):
                        pa.append(lambda o=o, cc=cc, f=(cc == 0):
                                  mm(o, cc, f, False))
                pb = [lambda: load(1)]
                for cc in range(1, HCH, 2):
                    for o in range(2):
                        pb.append(lambda o=o, cc=cc, l=(cc == HCH - 1):
                                  mm(o, cc, False, l))
                pb += [lambda: fin(0), lambda: fin(1)]
                return pa, pb

            def attn_block(b, qb, feed, split_ag=False, post_hp=None):
                def drain(n):
                    for _ in range(n):
                        if not feed.step():
                            return

                for hp in range(2):
                    ctxp = psC.tile([DH + 1, 2, TB], F32, tag="x",
                                    name=f"ctx_{b}_{qb}_{hp}")
                    kcs = list(range(4 * qb + 4))
                    for i, kc in enumerate(kcs):
                        c0 = 128 * (kc - 4 * qb) if kc >= 4 * qb else 0
                        nsl = slice(c0, TB)
                        qsl = slice(qb * TB + c0, (qb + 1) * TB)
                        sps = psS.tile([128, 2, TB], F32, tag="s",
                                       name=f"s_{b}_{qb}_{hp}_{kc}")
                        for x, off in ((0, 0), (1, 64)):
                            nc.tensor.matmul(
                                sps[:, x, nsl],
                                kT_sb[b][off:off + 64, kc * KC:(kc + 1) * KC],
                                qT_sb[b][hp][off:off + 64, qsl],
                                start=True, stop=True)
                        if kc >= 4 * qb:
                            for x in range(2):
                                nc.vector.tensor_tensor(
                                    sps[:, x, c0:c0 + 128],
                                    sps[:, x, c0:c0 + 128], mk_sb[:], ADD)
                        p = attn.tile([128, 2, TB], BF16, tag="p")
                        if EXP_MERGE:
                            nc.scalar.activation(p[:, :, nsl], sps[:, :, nsl],
                                                 EXPF, scale=SCALE)
                        else:
                            for x in range(2):
                                nc.scalar.activation(p[:, x, nsl],
                                                     sps[:, x, nsl],
                                                     EXPF, scale=SCALE)
                        for x in range(2):
                            nc.tensor.matmul(ctxp[:, x, nsl],
                                             v_sb[b][:, kc, 0:DH + 1],
                                             p[:, x, nsl], start=(i == 0),
                                             stop=(i == len(kcs) - 1),
                                             skip_group_check=True)
                        drain(2)
                    # normalize both heads of the pair; pack into one tile
                    ctxn = attn.tile([DH, 2, TB], BF16, tag="ctxn")
                    for x in range(2):
                        # fast approx reciprocal (~18 bits, 5x faster than
                        # DVE reciprocal; denom > 0 always so no edge cases).
                        # Stage the denominator row to SBUF via the Scalar
                        # engine (idle at block ends): the custom-DVE op
                        # misreads partition-64 PSUM sources directly.
                        rc = attn.tile([1, TB], F32, tag="rc")
                        dcp = attn.tile([1, TB], F32, tag="dcp")
                        nc.scalar.copy(dcp[:], ctxp[DH:DH + 1, x, :])
                        nc.vector.reciprocal_approx_fast(rc[:], dcp[:])
                        rb = attn.tile([DH, TB], F32, tag="rb")
                        nc.gpsimd.partition_broadcast(rb[:], rc[:])
                        nc.vector.tensor_tensor(ctxn[:, x, :],
                                                ctxp[0:DH, x, :], rb[:], MUL)
                        drain(2)
                    # two plain contiguous DMAs: the interleaved-rearrange
                    # single DMA cost 8.3us of Sync issue time
                    for x in range(2):
                        if split_ag:
                            nc.sync.dma_start(agi2[hp][64 * x:64 * x + 64, :],
                                              ctxn[:, x, :])
                        else:
                            r0 = 128 * hp + 64 * x
                            nc.sync.dma_start(agi[b][qb][r0:r0 + 64, :],
                                              ctxn[:, x, :])
                    if split_ag:
                        nc.gpsimd.collective_compute(
                            "AllGather", mybir.AluOpType.bypass,
                            replica_groups=[list(range(NC))],
                            ins=[agi2[hp].opt()], outs=[ago2[hp].opt()])
                    if post_hp is not None:
                        post_hp(hp)
                    drain(2)

            import concourse.mybir as _mybir

            class Feed:
                """Drain source for attn_block: first advances the next
                block's proj emission (so its PSUM-bank claims stay ahead of
                o_proj's), then pops queued o_proj tasks."""

                def __init__(self, gen, tasks):
                    self.gen = gen
                    self.tasks = tasks

                def step(self):
                    if self.gen is not None:
                        try:
                            next(self.gen)
                            return True
                        except StopIteration:
                            self.gen = None
                    if self.tasks:
                        self.tasks.pop(0)()
                        return True
                    return False

            # software pipeline: proj(n+1) emission is interleaved INTO
            # attn(n)'s drain slots (ahead of o_proj tasks), so the PE queue
            # always holds ready work while attn(n) waits on ACT.
            # o_proj(n) drains during attn(n+3) mid-stream; the last two
            # attention blocks each drain two o_projs so only the final
            # (split-AG) o_proj remains in the tail.
            blocks = [(b, qb) for b in range(B) for qb in range(QBS)]
            queue = []             # FIFO of oproj task lists
            for _ in proj_steps(*blocks[0], preload=load_consts):
                pass
            emit_vtrans(*blocks[0])
            load_wo()
            for i, (b, qb) in enumerate(blocks):
                gen = (proj_steps(*blocks[i + 1])
                       if i + 1 < len(blocks) else None)
                last = (i == len(blocks) - 1)
                npop = (2 if i >= 6 else (1 if i >= 3 else 0))
                loads, tasks = [], []
                while queue and npop > 0:
                    lst = queue.pop(0)
                    loads += lst[:4]     # hoist the 4 chunked c-loads
                    tasks += lst[4:]
                    npop -= 1
                tasks = loads + tasks
                feed = Feed(gen, tasks)
                if last:
                    pa, pb = oproj_split_tasks(b, qb)

                    def post_hp(hp):
                        if hp == 0:
                            feed.tasks += pa
                        else:
                            # issue the hp1 c-load now: the DMA parks on the
                            # AllGather semaphore and fires the moment the
                            # gathered data lands
                            pb.pop(0)()
                    attn_block(b, qb, feed, split_ag=True, post_hp=post_hp)
                else:
                    attn_block(b, qb, feed)
                # finish proj(n+1) emission (incl. its PE V-transpose)
                # BEFORE the AllGather so nothing serializes behind it
                if feed.gen is not None:
                    for _ in feed.gen:
                        pass
                    feed.gen = None
                if i + 1 < len(blocks):
                    emit_vtrans(*blocks[i + 1])
                if not last:
                    nc.gpsimd.collective_compute(
                        "AllGather", _mybir.AluOpType.bypass,
                        replica_groups=[list(range(NC))],
                        ins=[agi[b][qb].opt()], outs=[ago[b][qb].opt()])
                while feed.step():
                    pass
                if not last:
                    queue.append(oproj_tasks(b, qb))
                else:
                    for t in pb:
                        t()

    nc.compile()
    return nc


def _build_general():
    """Fallback f32 path for non-causal masks (full mask streamed)."""
    import concourse.mybir as mybir
    import concourse.tile as tile
    from concourse import bacc
    from concourse.masks import make_identity

    F32 = mybir.dt.float32
    F32R = mybir.dt.float32r
    EXPF = mybir.ActivationFunctionType.Exp
    ADD = mybir.AluOpType.add
    MUL = mybir.AluOpType.mult

    nc = bacc.Bacc("TRN2", target_bir_lowering=False, debug=False, num_devices=NC)

    hT = nc.dram_tensor("hT", [HID, T], F32R, kind="ExternalInput")
    wqT = nc.dram_tensor("wqT", [HID, CPC], F32R, kind="ExternalInput")
    wkvT = nc.dram_tensor("wkvT", [HID, 2 * DH], F32R, kind="ExternalInput")
    woT = nc.dram_tensor("woT", [H * DH, CPC], F32R, kind="ExternalInput")
    cosT = nc.dram_tensor("cosT", [DH, T], F32, kind="ExternalInput")
    sinT = nc.dram_tensor("sinT", [DH, T], F32, kind="ExternalInput")
    rotp = nc.dram_tensor("rotp", [DH, DH], F32R, kind="ExternalInput")
    maskg = nc.dram_tensor("maskg", [S, S], F32, kind="ExternalInput")
    outT = nc.dram_tensor("outT", [CPC, T], F32, kind="ExternalOutput")

    with tile.TileContext(nc) as tc:
        with tc.tile_pool(name="const", bufs=1) as cpool, \
             tc.tile_pool(name="big", bufs=1) as big, \
             tc.tile_pool(name="stream", bufs=3) as stream, \
             tc.tile_pool(name="rope", bufs=2) as rope, \
             tc.tile_pool(name="attn", bufs=3) as attn, \
             tc.tile_pool(name="psM", bufs=1, space="PSUM") as psM, \
             tc.tile_pool(name="psS", bufs=3, space="PSUM") as psS, \
             tc.tile_pool(name="psC", bufs=1, space="PSUM") as psC, \
             tc.tile_pool(name="dram", bufs=1, space="DRAM") as dram:

            wq_sb = cpool.tile([128, HCH, CPC], F32R)
            nc.sync.dma_start(wq_sb[:], wqT[:].rearrange("(o p) m -> p o m", p=128))
            wkv_sb = cpool.tile([128, HCH, 2 * DH], F32R)
            nc.sync.dma_start(wkv_sb[:], wkvT[:].rearrange("(o p) m -> p o m", p=128))
            wo_sb = cpool.tile([128, HCH, CPC], F32R)
            nc.sync.dma_start(wo_sb[:], woT[:].rearrange("(o p) m -> p o m", p=128))
            cos_sb = cpool.tile([DH, T], F32)
            nc.sync.dma_start(cos_sb[:], cosT[:])
            sin_sb = cpool.tile([DH, T], F32)
            nc.sync.dma_start(sin_sb[:], sinT[:])
            rot_sb = cpool.tile([DH, DH], F32R)
            nc.sync.dma_start(rot_sb[:], rotp[:])
            onesc_f = cpool.tile([128, SB_KC, 1], F32)
            nc.any.memset(onesc_f[:], 1.0)
            ident = cpool.tile([DH, DH], F32)
            make_identity(nc, ident)

            qT_sb = [[big.tile([128, S], F32R, tag=f"qT{b}{hp}", name=f"qT{b}{hp}")
                      for hp in range(2)] for b in range(B)]
            kT_sb = [big.tile([128, S], F32R, tag=f"kT{b}", name=f"kT{b}")
                     for b in range(B)]
            v_sb = [big.tile([128, SB_KC, DH + 1], F32R, tag=f"v{b}", name=f"v{b}")
                    for b in range(B)]
            for b in range(B):
                nc.vector.tensor_copy(v_sb[b][:, :, DH:DH + 1], onesc_f[:])

            ag_in = [dram.tile([CPC, S], F32R, name=f"agi{b}") for b in range(B)]
            ag_out = [dram.tile([H * DH, S], F32R, name=f"ago{b}",
                                addr_space="Shared") for b in range(B)]

            def proj_block(b, qb):
                tb = b * QBS + qb
                gs = slice(tb * TB, (tb + 1) * TB)
                ls = slice(qb * TB, (qb + 1) * TB)
                pq = [psM.tile([128, TB], F32, tag=f"mm{hp}", name=f"pq{hp}_{tb}")
                      for hp in range(2)]
                pkv = psM.tile([128, TB], F32, tag="mmkv")
                for cc in range(HCH):
                    h_sb = stream.tile([128, TB], F32R, tag="h")
                    nc.sync.dma_start(h_sb[:], hT[cc * 128:(cc + 1) * 128, gs])
                    for hp in range(2):
                        nc.tensor.matmul(pq[hp][:],
                                         wq_sb[:, cc, hp * 128:(hp + 1) * 128],
                                         h_sb[:], start=(cc == 0),
                                         stop=(cc == HCH - 1))
                    nc.tensor.matmul(pkv[:], wkv_sb[:, cc, :], h_sb[:],
                                     start=(cc == 0), stop=(cc == HCH - 1))
                for h in range(HPC):
                    hp, hh = h // 2, 64 * (h % 2)
                    src = pq[hp][hh:hh + 64, :]
                    qraw = rope.tile([DH, TB], F32R, tag="raw")
                    nc.vector.tensor_copy(qraw[:], src)
                    qcos = rope.tile([DH, TB], F32, tag="cos")
                    nc.vector.tensor_tensor(qcos[:], qraw[:].bitcast(F32),
                                            cos_sb[:, gs], MUL)
                    rps = psS.tile([DH, TB], F32, tag="s")
                    nc.tensor.matmul(rps[:], rot_sb[:], qraw[:], start=True,
                                     stop=True)
                    qsin = rope.tile([DH, TB], F32, tag="sin")
                    nc.vector.tensor_tensor(qsin[:], rps[:], sin_sb[:, gs], MUL)
                    if hh == 0:
                        nc.vector.tensor_tensor(qT_sb[b][hp][0:64, ls],
                                                qcos[:], qsin[:], ADD)
                    else:
                        qfin = rope.tile([DH, TB], F32R, tag="fin")
                        nc.vector.tensor_tensor(qfin[:], qcos[:], qsin[:], ADD)
                        nc.sync.dma_start(qT_sb[b][hp][64:128, ls], qfin[:])
                ksrc = pkv[64:128, :]
                kraw = rope.tile([DH, TB], F32R, tag="raw")
                nc.vector.tensor_copy(kraw[:], ksrc)
                kcos = rope.tile([DH, TB], F32, tag="cos")
                nc.vector.tensor_tensor(kcos[:], kraw[:].bitcast(F32),
                                        cos_sb[:, gs], MUL)
                krps = psS.tile([DH, TB], F32, tag="s")
                nc.tensor.matmul(krps[:], rot_sb[:], kraw[:], start=True, stop=True)
                ksin = rope.tile([DH, TB], F32, tag="sin")
                nc.vector.tensor_tensor(ksin[:], krps[:], sin_sb[:, gs], MUL)
                nc.vector.tensor_tensor(kT_sb[b][0:64, ls], kcos[:], ksin[:], ADD)
                nc.sync.dma_start(kT_sb[b][64:128, ls], kT_sb[b][0:64, ls])
                vraw = rope.tile([DH, TB], F32, tag="vraw")
                nc.vector.tensor_copy(vraw[:], pkv[0:64, :])
                for i in range(TB // KC):
                    vtp = psS.tile([128, DH], F32, tag="s")
                    nc.tensor.transpose(vtp[:], vraw[:, i * KC:(i + 1) * KC],
                                        ident[:])
                    nc.vector.tensor_copy(v_sb[b][:, qb * (TB // KC) + i, 0:DH],
                                          vtp[:])

            def attn_block(b, qb):
                for hp in range(2):
                    kcs = list(range(SB_KC))
                    ctxp = [psC.tile([DH + 1, TB], F32, tag=f"ctx{x}",
                                     name=f"ctx{x}_{b}_{qb}_{hp}")
                            for x in range(2)]
                    for i, kc in enumerate(kcs):
                        sps = [psS.tile([128, TB], F32, tag="s",
                                        name=f"s{x}_{b}_{qb}_{hp}_{kc}")
                               for x in range(2)]
                        for x, hh in enumerate((0, 64)):
                            nc.tensor.matmul(
                                sps[x][:],
                                kT_sb[b][hh:hh + 64, kc * KC:(kc + 1) * KC],
                                qT_sb[b][hp][hh:hh + 64,
                                             qb * TB:(qb + 1) * TB],
                                start=True, stop=True)
                        for x in range(2):
                            mg = attn.tile([128, TB], F32, tag="mg")
                            nc.sync.dma_start(
                                mg[:], maskg[kc * KC:(kc + 1) * KC,
                                             qb * TB:(qb + 1) * TB])
                            nc.vector.tensor_tensor(sps[x][:], sps[x][:],
                                                    mg[:], ADD)
                            p_sb = attn.tile([128, TB], F32R, tag="p")
                            nc.scalar.activation(p_sb[:], sps[x][:],
                                                 EXPF, scale=SCALE)
                            nc.tensor.matmul(ctxp[x][:], v_sb[b][:, kc, :],
                                             p_sb[:], start=(i == 0),
                                             stop=(i == len(kcs) - 1),
                                             skip_group_check=True)
                    for x in range(2):
                        h = 2 * hp + x
                        rc = attn.tile([1, TB], F32R, tag="rc")
                        with nc.allow_low_precision(reason="f32r rounding ~1e-4"):
                            nc.vector.reciprocal(rc[:], ctxp[x][DH:DH + 1, :])
                        rb = attn.tile([DH, TB], F32R, tag="rb")
                        nc.gpsimd.partition_broadcast(rb[:], rc[:])
                        ctxn = attn.tile([DH, TB], F32R, tag="ctxn")
                        nc.vector.tensor_tensor(ctxn[:], ctxp[x][0:DH, :],
                                                rb[:].bitcast(F32), MUL)
                        nc.sync.dma_start(
                            ag_in[b][h * DH:(h + 1) * DH,
                                     qb * TB:(qb + 1) * TB],
                            ctxn[:])

            for b in range(B):
                for qb in range(QBS):
                    proj_block(b, qb)
                    attn_block(b, qb)
                import concourse.mybir as _mybir
                nc.gpsimd.collective_compute(
                    "AllGather", _mybir.AluOpType.bypass,
                    replica_groups=[list(range(NC))],
                    ins=[ag_in[b].opt()], outs=[ag_out[b].opt()])

            for b in range(B):
                for qb in range(QBS):
                    ls = slice(qb * TB, (qb + 1) * TB)
                    gs = slice((b * QBS + qb) * TB, (b * QBS + qb + 1) * TB)
                    po = [psM.tile([128, TB], F32, tag=f"mm{o}",
                                   name=f"po{o}_{b}_{qb}")
                          for o in range(2)]
                    for cc in range(HCH):
                        c_sb = stream.tile([128, TB], F32R, tag="c")
                        nc.sync.dma_start(
                            c_sb[:], ag_out[b][cc * 128:(cc + 1) * 128, ls])
                        for o in range(2):
                            nc.tensor.matmul(po[o][:],
                                             wo_sb[:, cc, o * 128:(o + 1) * 128],
                                             c_sb[:], start=(cc == 0),
                                             stop=(cc == HCH - 1))
                    for o in range(2):
                        o_sb = stream.tile([128, TB], F32, tag="o")
                        nc.vector.tensor_copy(o_sb[:], po[o][:])
                        nc.sync.dma_start(outT[o * 128:(o + 1) * 128, gs],
                                          o_sb[:])

    nc.compile()
    return nc


def _host_inputs(hidden_states, cos, sin, attention_mask, Wq, Wk, Wv, Wo,
                 causal):
    hT = np.ascontiguousarray(hidden_states.reshape(T, HID).T)
    cosT = np.ascontiguousarray(cos.reshape(T, DH).T)
    sinT = np.ascontiguousarray(sin.reshape(T, DH).T)
    # rot_half as a signed permutation: rot[d] = -x[d+32] (d<32), +x[d-32]
    p64 = np.zeros((DH, DH), np.float32)
    for m in range(32):
        p64[m + 32, m] = -1.0
        p64[m, m + 32] = 1.0
    WqT = np.ascontiguousarray(Wq.T)      # [HID, H*DH]
    WkT = np.ascontiguousarray(Wk.T)      # [HID, HKV*DH]
    WvT = np.ascontiguousarray(Wv.T)
    WoT = np.ascontiguousarray(Wo.T)      # [H*DH, HID]

    if causal:
        import ml_dtypes
        bf = ml_dtypes.bfloat16
        # partition-major pre-arrangements: X[o*128+p, m] -> X3[p, o, m]
        # (and per-block for hT) so each device DMA is one contiguous run
        # per partition
        hT = np.ascontiguousarray(
            hT.astype(bf).reshape(HCH, 128, B * S // 512, 512)
            .transpose(1, 2, 0, 3))
        WqT, WkT, WvT, WoT = (w.astype(bf) for w in (WqT, WkT, WvT, WoT))
        cosT = cosT.astype(bf)
        sinT = sinT.astype(bf)
        # block-diag rot for head-paired RoPE ([128,128]); upper-left 64x64
        # block doubles as the single-head (K) rot matrix
        p128 = np.zeros((128, 128), np.float32)
        p128[0:DH, 0:DH] = p64
        p128[DH:128, DH:128] = p64
        p128 = p128.astype(bf)
    def pmaj(w):
        """[HCH*128, m] -> [128, HCH, m] partition-major (causal path)."""
        if not causal:
            return np.ascontiguousarray(w)
        return np.ascontiguousarray(
            w.reshape(HCH, 128, w.shape[1]).transpose(1, 0, 2))

    ins = []
    for c in range(NC):
        d = {
            "hT": hT,
            "wqT": pmaj(WqT[:, c * CPC:(c + 1) * CPC]),
            "wkvT": pmaj(
                np.concatenate([WvT[:, c * DH:(c + 1) * DH],
                                WkT[:, c * DH:(c + 1) * DH]], axis=1)),
            "woT": pmaj(WoT[:, c * CPC:(c + 1) * CPC]),
            "cosT": cosT, "sinT": sinT,
        }
        if causal:
            d["rot2p"] = p128
            d["id64"] = np.eye(DH, dtype=np.float32).astype(p128.dtype)
        else:
            d["rotp"] = p64
        if causal:
            i = np.arange(128, dtype=np.float32)[:, None]
            cc = np.arange(128, dtype=np.float32)[None, :]
            d["maskd"] = np.where(cc < i, NEG, 0.0).astype(np.float32)
        else:
            m = attention_mask[0, 0].astype(np.float32)
            d["maskg"] = np.ascontiguousarray(m.T) * np.float32(1.0 / SCALE)
        ins.append(d)
    return ins


def _is_causal(attention_mask):
    if attention_mask.shape != (1, 1, S, S):
        return False
    m = attention_mask[0, 0]
    neg = np.finfo(np.float32).min
    tril = np.tril(np.ones((S, S), dtype=bool))
    expect = np.where(tril, np.float32(0.0), np.float32(neg))
    return np.array_equal(m, expect)


_CACHE = {}


def _get_nc(causal):
    if causal not in _CACHE:
        _CACHE[causal] = _build_fast() if causal else _build_general()
    return _CACHE[causal]


def kernel(**inputs) -> np.ndarray:
    from concourse.bass_utils import run_bass_kernel_spmd

    hidden_states = np.asarray(inputs["hidden_states"], np.float32)
    cos = np.asarray(inputs["cos"], np.float32)
    sin = np.asarray(inputs["sin"], np.float32)
    attention_mask = np.asarray(inputs["attention_mask"], np.float32)
    Wq = np.asarray(inputs["Wq"], np.float32)
    Wk = np.asarray(inputs["Wk"], np.float32)
    Wv = np.asarray(inputs["Wv"], np.float32)
    Wo = np.asarray(inputs["Wo"], np.float32)

    causal = _is_causal(attention_mask)
    nc = _get_nc(causal)
    ins = _host_inputs(hidden_states, cos, sin, attention_mask,
                       Wq, Wk, Wv, Wo, causal)
    res = run_bass_kernel_spmd(nc, ins, core_ids=list(range(NC)))
    outT = np.concatenate([res.results[c]["outT"] for c in range(NC)], axis=0)
    return np.ascontiguousarray(outT.T).reshape(B, S, HID)

